# revision 16
# baseline (speedup 1.0000x reference)
"""TransformerXL relative attention on 8 TRN2 NeuronCores — v3.

Sharding: TP over heads 4-way x DP over batch 2-way.  Core c handles
batch group bg=c//4 (batches 2bg, 2bg+1) and head group hg=c%4 (4 heads,
256 head-dims).  Each core computes a partial output projection
[CUR, 2, DIM]; the host sums the 4 head-group partials per batch.

v3 vs v2 (129.3us):
- TP4xDP2 instead of TP2xDP4: the batch-independent R^T = pos @ W_pos
  projection halves (2 t-tiles instead of 4), saving 16384 PE cycles;
  everything else is work-neutral (8 (head,batch) attention units per
  core either way).
- The +u / +v query biases are folded into the Q eviction as DVE
  tensor_scalar adds (per-partition scalar AP), removing the PE
  ones-row matmuls (-4096 cycles).
- The shifted position-score readback is one 3-D AP accum DMA per unit
  ([[RSTR,128],[128*RSTR,4],[1,1024]]) instead of 4 per-qt reads,
  cutting SWDGE descriptor-gen on GPSIMD from 33us to 9us.
"""

import numpy as np
import ml_dtypes

import concourse.bass as bass
import concourse.mybir as mybir
import concourse.tile as tile
from concourse import bacc
from concourse.bass_utils import run_bass_kernel_spmd
from concourse.masks import make_identity

CUR, FULL, BS, DIM, H, D = 512, 1024, 4, 1024, 16, 64
NHC = 4                 # heads per core
NB = 2                  # batches per core
HDC = NHC * D           # 256 head-dims per core
SCALE = 1.0 / D ** 0.5  # 0.125
BIG = -30000.0
PADW = 1536             # padded row width for the shift round trip
RSTR = PADW - 1         # shifted read row stride
BF = mybir.dt.bfloat16
F32 = mybir.dt.float32
Exp = mybir.ActivationFunctionType.Exp
Copy = mybir.ActivationFunctionType.Copy
AluAdd = mybir.AluOpType.add

_CACHED = {}


def build_program():
    nc = bacc.Bacc(None, target_bir_lowering=False, debug=False)
    # One packed input tensor, phase-ordered so a handful of big DMAs
    # stream it in the order the projection loops consume it:
    #   R-block  8 k8-slices of (xpos 1024 | wpos 256)       = 8*1280
    #   Q-block  8 k8-slices of (xcur 2b*512 | wq 256)        = 8*1280
    #   K-block  8 k8-slices of (xfull 2b*1024 | wk 256)      = 8*2304
    #   V-block  8 k8-slices of (wv 256)                      = 8*256
    #   wproj    hd-major [128, 2, 1024]                      = 2048
    PCOLS = 8 * 1280 + 8 * 1280 + 8 * 2304 + 8 * 256 + 2048
    packed = nc.declare_dram_parameter("packed", [128, PCOLS], BF,
                                       isOutput=False)
    uvp_in = nc.declare_dram_parameter("uvp", [128, 4], F32, isOutput=False)
    outp = nc.declare_dram_parameter("outp", [CUR, NB * DIM], F32, isOutput=True)
    R0, Q0, K0, V0, W0 = (0, 8 * 1280, 8 * 2560, 8 * 2560 + 8 * 2304,
                          8 * 2560 + 8 * 2304 + 8 * 256)

    with tile.TileContext(nc) as tc:
        const = tc.alloc_tile_pool(name="const", bufs=1)
        psA = tc.alloc_tile_pool(name="psA", bufs=3, space="PSUM")
        psB = tc.alloc_tile_pool(name="psB", bufs=2, space="PSUM")
        psT = tc.alloc_tile_pool(name="psT", bufs=2, space="PSUM")
        psO = tc.alloc_tile_pool(name="psO", bufs=1, space="PSUM")
        slabp = tc.alloc_tile_pool(name="slabp", bufs=3)
        sp = tc.alloc_tile_pool(name="sp", bufs=3)
        atp = tc.alloc_tile_pool(name="atp", bufs=3)
        work = tc.alloc_tile_pool(name="work", bufs=2)
        dram = tc.alloc_tile_pool(name="dram", bufs=4, space="DRAM")

        # ---- resident SBUF tensors ----
        # pk_sb mirrors the packed DRAM layout 1:1
        pk_sb = const.tile([128, PCOLS], BF)
        uvp_sb = const.tile([128, 4], F32)
        ident = const.tile([128, 128], BF)
        big_sb = const.tile([128, 512], BF)
        kt_sb = const.tile([128, NB, 2, FULL], BF)   # K^T (dc, b, t, j)
        rt_sb = const.tile([128, 2, FULL], BF)       # R^T (dc, t, m)
        v_sb = const.tile([128, 8, NB, NHC, D + 1], BF)
        qt_sb = const.tile([128, NB, 2, 2, CUR], BF)  # Q^T +u/+v (dc,b,t,uv,i)
        o_sb = const.tile([128, NB, 2, CUR], BF)      # O^T normalized

        # packed views (k8-sliced)
        def xpos_v(k8):
            return pk_sb[:, R0 + k8 * 1280: R0 + k8 * 1280 + 1024]

        def wpos_v(k8, t):
            c = R0 + k8 * 1280 + 1024 + t * 128
            return pk_sb[:, c:c + 128]

        def xcur_v(k8, b):
            c = Q0 + k8 * 1280 + b * 512
            return pk_sb[:, c:c + 512]

        def wq_v(k8, t):
            c = Q0 + k8 * 1280 + 1024 + t * 128
            return pk_sb[:, c:c + 128]

        def xfull_v(k8, b, c0, c1):
            c = K0 + k8 * 2304 + b * 1024
            return pk_sb[:, c + c0:c + c1]

        def wk_v(k8, t):
            c = K0 + k8 * 2304 + 2048 + t * 128
            return pk_sb[:, c:c + 128]

        def wv_v(k8):
            c = V0 + k8 * 256
            return pk_sb[:, c:c + 256]

        def wproj_v(t, c0, c1):
            c = W0 + t * 1024
            return pk_sb[:, c + c0:c + c1]

        # phase-ordered streaming loads: few, large DMAs
        nc.sync.dma_start(out=uvp_sb[:], in_=uvp_in[:])
        bounds = [R0, R0 + 1280, R0 + 2 * 1280, R0 + 4 * 1280, R0 + 6 * 1280,
                  Q0, Q0 + 4 * 1280,
                  K0, K0 + 2304, K0 + 2 * 2304, K0 + 3 * 2304, K0 + 4 * 2304,
                  K0 + 5 * 2304, K0 + 6 * 2304, K0 + 7 * 2304,
                  V0, PCOLS]
        for c0, c1 in zip(bounds[:-1], bounds[1:]):
            nc.sync.dma_start(out=pk_sb[:, c0:c1], in_=packed[:, c0:c1])
        make_identity(nc, ident[:])
        nc.vector.memset(big_sb[:], BIG)
        nc.gpsimd.memset(v_sb[:, :, :, :, D:D + 1], 1.0)

        # shift buffers: one per unit slot (double-buffered across units);
        # pad columns [1024,1536) hold BIG once (causal mask for free).
        pdram = []
        for rep in range(4):
            t = dram.tile([CUR * PADW], BF, tag=f"pd_{rep}", name=f"pd_{rep}")
            pdram.append(t)
            for rq in range(4):
                nc.sync.dma_start(
                    out=bass.AP(tensor=t.tensor,
                                offset=rq * 128 * PADW + FULL,
                                ap=[[PADW, 128], [1, 512]]),
                    in_=big_sb[:])

        # ---- projections ----
        def stage_r_start():
            """All 4 R^T groups k8-major across 4 PSUM tiles, chasing the
            per-k8 input DMA slices."""
            tiles = [psA.tile([128, 512], F32, tag="pj", name=f"p_r0_{g}")
                     for g in range(2)]
            tiles += [psB.tile([128, 512], F32, tag="pv", name=f"p_r1_{g}")
                      for g in range(2)]
            for k8 in range(8):
                for g, pk in enumerate(tiles):
                    t, nh = g // 2, g % 2
                    nc.tensor.matmul(
                        pk[:], wpos_v(k8, t),
                        xpos_v(k8)[:, nh * 512:(nh + 1) * 512],
                        start=(k8 == 0), stop=(k8 == 7))
            for g, pk in enumerate(tiles):
                t, nh = g // 2, g % 2
                nc.vector.tensor_copy(
                    rt_sb[:, t, nh * 512:(nh + 1) * 512], pk[:])

        def stage_q():
            """Q k8-major across 4 PSUM tiles, chasing the xcur slices."""
            tiles = [psA.tile([128, 512], F32, tag="pj", name=f"p_q_{g}")
                     for g in range(2)]
            tiles += [psB.tile([128, 512], F32, tag="pv", name=f"p_q1_{g}")
                      for g in range(2)]
            for k8 in range(8):
                for g, pq in enumerate(tiles):
                    b, t = g // 2, g % 2
                    nc.tensor.matmul(
                        pq[:], wq_v(k8, t), xcur_v(k8, b),
                        start=(k8 == 0), stop=(k8 == 7))
            for g, pq in enumerate(tiles):
                b, t = g // 2, g % 2
                nc.vector.tensor_scalar_add(
                    qt_sb[:, b, t, 0, :], pq[:], uvp_sb[:, 2 * t:2 * t + 1])
                nc.vector.tensor_scalar_add(
                    qt_sb[:, b, t, 1, :], pq[:],
                    uvp_sb[:, 2 * t + 1:2 * t + 2])

        def pos_unit(u):
            for qt in range(4):
                a_pos_qt(u, qt)

        def stage_k(b):
            """K groups sequential (xfull slices stream in during R/Q)."""
            if True:
                for t in range(2):
                    for nh in range(2):
                        pk = psA.tile([128, 512], F32, tag="pj",
                                      name=f"p_k_{b}{t}{nh}")
                        for k8 in range(8):
                            nc.tensor.matmul(
                                pk[:], wk_v(k8, t),
                                xfull_v(k8, b, nh * 512, (nh + 1) * 512),
                                start=(k8 == 0), stop=(k8 == 7))
                        nc.vector.tensor_copy(
                            kt_sb[:, b, t, nh * 512:(nh + 1) * 512], pk[:])

        def v_group(b, jt):
            pv = psB.tile([128, HDC], F32, tag="pv", name=f"p_v_{b}{jt}")
            for k8 in range(8):
                nc.tensor.matmul(
                    pv[:], xfull_v(k8, b, jt * 128, (jt + 1) * 128),
                    wv_v(k8),
                    start=(k8 == 0), stop=(k8 == 7))
            nc.vector.tensor_copy(
                v_sb[:, jt, b, :, 0:D],
                pv[:].rearrange("p (h d) -> p h d", h=NHC))

        # ---- attention stages (per unit u = (h, b)) ----
        s_tiles = {}    # u -> content+shifted-pos scores [128, 4, FULL]
        at_tiles = {}   # u -> [128, 8, CUR] A^T blocks
        slab_tiles = {}
        ov_tiles = {}

        def hb(u):
            return u % 4, u // 4

        def evict(engine, out, in_):
            if engine == "v":
                nc.vector.tensor_copy(out, in_)
            else:
                nc.scalar.activation(out, in_, Copy)

        SLAB_ENG = {qt: "v" for qt in range(4)}
        CONT_ENG = {(0, 0): "a", (1, 0): "a", (2, 0): "a", (3, 0): "a",
                    (0, 1): "a", (1, 1): "a", (2, 1): "v", (3, 1): "v"}

        def a_pos_qt(u, qt):
            """Position scores for one query tile: matmul, evict, write."""
            h, b = hb(u)
            p0 = (h % 2) * 64
            th = h // 2
            if qt == 0:
                slab_tiles[u] = slabp.tile([128, 4, FULL], BF, tag="slab",
                                           name=f"slab_{u}")
            slab = slab_tiles[u]
            m_min = 384 - 128 * qt
            c = m_min
            ci = 0
            while c < 1024:
                ce = min(c + 512, 1024)
                pp = psA.tile([128, ce - c], F32, tag="pj",
                              name=f"pp_{u}_{qt}_{ci}")
                nc.tensor.matmul(
                    pp[:],
                    qt_sb[p0:p0 + 64, b, th, 1, qt * 128:(qt + 1) * 128],
                    rt_sb[p0:p0 + 64, th, c:ce],
                    start=True, stop=True)
                evict(SLAB_ENG[qt], slab[:, qt, c:ce], pp[:])
                c = ce
                ci += 1
            pd = pdram[u % 4]
            nc.sync.dma_start(
                out=bass.AP(tensor=pd.tensor,
                            offset=qt * 128 * PADW + m_min,
                            ap=[[PADW, 128], [1, 1024 - m_min]]),
                in_=slab[:, qt, m_min:1024])

        def a_content_qt(u, qt):
            """Content scores for one tile + shifted-pos accumulate DMA."""
            h, b = hb(u)
            p0 = (h % 2) * 64
            th = h // 2
            pd = pdram[u % 4]
            if qt == 0:
                s_tiles[u] = sp.tile([128, 4, FULL], BF, tag="s", name=f"s_{u}")
            s_all = s_tiles[u]
            jw = 640 + 128 * qt
            c = 0
            ci = 0
            while c < jw:
                ce = min(c + 512, jw)
                pc = psB.tile([128, ce - c], F32, tag="pv",
                              name=f"pc_{u}_{qt}_{ci}")
                nc.tensor.matmul(
                    pc[:],
                    qt_sb[p0:p0 + 64, b, th, 0, qt * 128:(qt + 1) * 128],
                    kt_sb[p0:p0 + 64, b, th, c:ce],
                    start=True, stop=True)
                evict(CONT_ENG[(qt, ci)], s_all[:, qt, c:ce], pc[:])
                c = ce
                ci += 1
            # shifted position rows accumulate onto the content scores
            nc.gpsimd.dma_start(
                out=s_all[:, qt, 0:jw],
                in_=bass.AP(tensor=pd.tensor,
                            offset=qt * 128 * PADW + 511 - 128 * qt,
                            ap=[[RSTR, 128], [1, jw]]),
                accum_op=AluAdd)

        def e_qt(u, qt, tpool=None):
            """Transpose hull blocks of one tile, exponentiate into A^T."""
            if qt == 0:
                at_tiles[u] = atp.tile([128, 8, CUR], BF, tag="at",
                                       name=f"at_{u}")
            at_all = at_tiles[u]
            s_all = s_tiles[u]
            nj8 = qt + 5
            st = (tpool or psT).tile([128, 8, 128], BF,
                                     tag="pj" if tpool else "pt",
                                     name=f"st_{u}_{qt}")
            for j8 in range(nj8):
                nc.tensor.transpose(st[:, j8, :],
                                    s_all[:, qt, j8 * 128:(j8 + 1) * 128],
                                    ident[:])
            nc.scalar.activation(
                at_all[:, 0:nj8, qt * 128:(qt + 1) * 128],
                st[:, 0:nj8, :], Exp, scale=SCALE)

        def f_av(u, jts, pool=None):
            """Part of A^T @ V accumulation (ones column -> denominator)."""
            h, b = hb(u)
            if jts[0] == 0:
                ov_tiles[u] = (pool or psO).tile(
                    [D + 1, CUR], F32, tag="pv" if pool else "po",
                    name=f"ov_{u}")
            ov = ov_tiles[u]
            at_all = at_tiles[u]
            for jt in jts:
                c0 = max(0, (jt - 4) * 128)
                nc.tensor.matmul(ov[:, c0:], v_sb[:, jt, b, h, :],
                                 at_all[:, jt, c0:],
                                 start=(jt == 0), stop=(jt == 7),
                                 skip_group_check=True)

        def f_norm(u, chunks=((0, CUR),), done=True):
            """Normalize by the softmax denominator into O^T."""
            h, b = hb(u)
            p0 = (h % 2) * 64
            th = h // 2
            ov = ov_tiles[u]
            if done:
                ov_tiles.pop(u)
                at_tiles.pop(u, None)
            for c0, c1 in chunks:
                rden = work.tile([1, CUR], F32, tag="rden", bufs=2,
                                 name=f"rden_{u}_{c0}")
                nc.vector.reciprocal(rden[0:1, 0:c1 - c0], ov[D:D + 1, c0:c1])
                rdb = work.tile([64, CUR], F32, tag="rdb", bufs=2,
                                name=f"rdb_{u}_{c0}")
                nc.gpsimd.partition_broadcast(rdb[0:64, 0:c1 - c0],
                                              rden[0:1, 0:c1 - c0])
                nc.vector.tensor_mul(o_sb[p0:p0 + 64, b, th, c0:c1],
                                     ov[0:D, c0:c1], rdb[0:64, 0:c1 - c0])

        def unit_slot(up, uc, ue, vgs=(), projqts=()):
            """One pipeline slot: position scores for unit up, content
            scores for unit uc, transpose/exp/AV for unit ue, plus
            V-projection or output-projection fillers."""
            vit = iter(vgs)
            pit = iter(projqts)
            for qt in range(4):
                if ue is not None:
                    e_qt(ue, qt)
                v = next(vit, None)
                if v is not None:
                    v_group(*v)
                if up is not None:
                    a_pos_qt(up, qt)
                if qt % 2 == 1:
                    p = next(pit, None)
                    if p is not None:
                        proj_qt(*p, pools=((psA, "pj"), (psA, "pj")))
            for qt in range(4):
                v = next(vit, None)
                if v is not None:
                    v_group(*v)
                if uc is not None:
                    a_content_qt(uc, qt)
                if ue is not None and qt % 2 == 1:
                    f_av(ue, ((qt - 1) * 2, (qt - 1) * 2 + 1,
                              (qt - 1) * 2 + 2, (qt - 1) * 2 + 3))
            if ue is not None:
                f_norm(ue)

        # ---- output projection ----
        proj_tiles = {}

        def proj_start(b, qt, pools=None, nhs=(0, 1)):
            for nh in nhs:
                pool, tg = (pools or ((psB, "pv"), (psO, "po")))[nh]
                pr = pool.tile([128, 512], F32, tag=tg, name=f"pr_{b}{qt}{nh}")
                proj_tiles[(b, qt, nh)] = pr
                nc.tensor.matmul(
                    pr[:], o_sb[:, b, 0, qt * 128:(qt + 1) * 128],
                    wproj_v(0, nh * 512, (nh + 1) * 512),
                    start=True, stop=False)

        def proj_finish(b, qt):
            ot = work.tile([128, FULL], F32, tag="ot", bufs=2,
                           name=f"ot_{b}{qt}")
            for nh in range(2):
                if (b, qt, nh) not in proj_tiles:
                    proj_start(b, qt, nhs=(nh,))
                pr = proj_tiles.pop((b, qt, nh))
                nc.tensor.matmul(
                    pr[:], o_sb[:, b, 1, qt * 128:(qt + 1) * 128],
                    wproj_v(1, nh * 512, (nh + 1) * 512),
                    start=False, stop=True)
                evict("a" if nh == 0 else "v",
                      ot[:, nh * 512:(nh + 1) * 512], pr[:])
                nc.sync.dma_start(
                    out=outp[qt * 128:(qt + 1) * 128,
                             b * DIM + nh * 512:b * DIM + (nh + 1) * 512],
                    in_=ot[:, nh * 512:(nh + 1) * 512])

        def proj_qt(b, qt, pools=None):
            proj_start(b, qt, pools=pools)
            proj_finish(b, qt)

        # ---- schedule ----
        # 3-phase software pipeline (pos / content / ef decoupled).
        # pos(0..3) run right after R+Q, inside the DMA-bound startup
        # window, before the xfull-dependent K/V matmuls enter the PE
        # queue; content follows per-batch K; ef trails by 2 slots.
        vlist = [(b, jt) for b in range(NB) for jt in range(8)]
        stage_r_start()
        stage_q()
        for u in range(4):
            pos_unit(u)
        stage_k(0)
        unit_slot(None, 0, None, vgs=vlist[0:4])
        stage_k(1)
        unit_slot(None, 1, None, vgs=vlist[4:8])
        unit_slot(4, 2, 0, vgs=vlist[8:12])
        unit_slot(5, 3, 1, vgs=vlist[12:16])
        unit_slot(6, 4, 2)
        unit_slot(7, 5, 3)
        unit_slot(None, 6, 4, projqts=((0, 0), (0, 1)))
        unit_slot(None, 7, 5, projqts=((0, 2), (0, 3)))
        # tail: interleave the last two units (both batch 1)
        u6, u7 = 6, 7
        for qt in range(4):
            e_qt(u6, qt)
        e_qt(u7, 0)
        e_qt(u7, 1)
        f_av(u6, (0, 1, 2, 3))
        e_qt(u7, 2)
        f_av(u6, (4, 5, 6, 7))
        e_qt(u7, 3)
        proj_start(1, 0, pools=((psA, "pj"), (psA, "pj")))
        f_norm(u6)
        f_av(u7, (0, 1, 2, 3), pool=psB)
        f_av(u7, (4, 5, 6, 7), pool=psB)

        for qt in range(4):
            f_norm(u7, chunks=((qt * 128, (qt + 1) * 128),), done=(qt == 3))
            if qt == 0:
                proj_finish(1, 0)
            else:
                proj_qt(1, qt)

        for p in (dram, work, atp, sp, slabp, psO, psT, psB, psA, const):
            p.release()
    nc.compile()
    return nc


def kernel(inputs, pos_embedding, full_input, u, v, W_kv, b_kv, W_q, b_q,
           W_pos, b_pos, W_proj, b_proj, mask):
    bf = ml_dtypes.bfloat16
    inputs = np.asarray(inputs)
    full_input = np.asarray(full_input)
    pos = np.asarray(pos_embedding)[:, 0, :]

    if "nc" not in _CACHED:
        _CACHED["nc"] = build_program()
    nc = _CACHED["nc"]

    PCOLS = 8 * 1280 + 8 * 1280 + 8 * 2304 + 8 * 256 + 2048
    R0, Q0, K0, V0, W0 = (0, 8 * 1280, 8 * 2560, 8 * 2560 + 8 * 2304,
                          8 * 2560 + 8 * 2304 + 8 * 256)
    posT = pos.T.astype(bf)                      # [DIM, FULL]
    in_maps = []
    for c in range(8):
        bg, hg = c // 4, c % 4
        sl = slice(hg * HDC, (hg + 1) * HDC)
        bsl = slice(2 * bg, 2 * bg + 2)
        uvec = (np.asarray(u).reshape(-1) + np.asarray(b_q))[sl]
        vvec = (np.asarray(v).reshape(-1) + np.asarray(b_q))[sl]
        # uvp[p, 2t+0/1] = u/v for head-dim t*128+p
        uvp = np.stack([uvec[0:128], vvec[0:128],
                        uvec[128:256], vvec[128:256]], axis=1)
        xf = full_input[:, bsl, :].transpose(2, 1, 0).astype(bf)  # [DIM,2,FULL]
        xc = inputs[:, bsl, :].transpose(2, 1, 0).astype(bf)      # [DIM,2,CUR]
        wq = W_q[:, sl].astype(bf)
        wk = W_kv[:, hg * HDC:(hg + 1) * HDC].astype(bf)
        wv = W_kv[:, H * D + hg * HDC:H * D + (hg + 1) * HDC].astype(bf)
        wpos = W_pos[:, sl].astype(bf)
        wproj = W_proj[sl, :].astype(bf)          # [HDC, DIM]

        P = np.zeros((128, PCOLS), bf)
        for k8 in range(8):
            dsl = slice(k8 * 128, (k8 + 1) * 128)
            P[:, R0 + k8 * 1280:R0 + k8 * 1280 + 1024] = posT[dsl]
            P[:, R0 + k8 * 1280 + 1024:R0 + (k8 + 1) * 1280] = wpos[dsl]
            P[:, Q0 + k8 * 1280:Q0 + k8 * 1280 + 1024] = \
                xc[dsl].reshape(128, NB * CUR)
            P[:, Q0 + k8 * 1280 + 1024:Q0 + (k8 + 1) * 1280] = wq[dsl]
            P[:, K0 + k8 * 2304:K0 + k8 * 2304 + 2048] = \
                xf[dsl].reshape(128, NB * FULL)
            P[:, K0 + k8 * 2304 + 2048:K0 + (k8 + 1) * 2304] = wk[dsl]
            P[:, V0 + k8 * 256:V0 + (k8 + 1) * 256] = wv[dsl]
        P[:, W0:W0 + 1024] = wproj[0:128]
        P[:, W0 + 1024:W0 + 2048] = wproj[128:256]
        in_maps.append({
            "packed": P,
            "uvp": np.ascontiguousarray(uvp).astype(np.float32),
        })

    _CACHED["maps"] = in_maps
    res = run_bass_kernel_spmd(nc, in_maps, list(range(8)))
    out = np.zeros((CUR, BS, DIM), np.float32)
    for c in range(8):
        bg, hg = c // 4, c % 4
        r = res.results[c]["outp"].reshape(CUR, NB, DIM)
        out[:, 2 * bg, :] += r[:, 0, :]
        out[:, 2 * bg + 1, :] += r[:, 1, :]
    return out


# revision 18
# speedup vs baseline: 1.0006x; 1.0006x over previous
"""TransformerXL relative attention on 8 TRN2 NeuronCores — v3.

Sharding: TP over heads 4-way x DP over batch 2-way.  Core c handles
batch group bg=c//4 (batches 2bg, 2bg+1) and head group hg=c%4 (4 heads,
256 head-dims).  Each core computes a partial output projection
[CUR, 2, DIM]; the host sums the 4 head-group partials per batch.

v3 vs v2 (129.3us):
- TP4xDP2 instead of TP2xDP4: the batch-independent R^T = pos @ W_pos
  projection halves (2 t-tiles instead of 4), saving 16384 PE cycles;
  everything else is work-neutral (8 (head,batch) attention units per
  core either way).
- The +u / +v query biases are folded into the Q eviction as DVE
  tensor_scalar adds (per-partition scalar AP), removing the PE
  ones-row matmuls (-4096 cycles).
- The shifted position-score readback is one 3-D AP accum DMA per unit
  ([[RSTR,128],[128*RSTR,4],[1,1024]]) instead of 4 per-qt reads,
  cutting SWDGE descriptor-gen on GPSIMD from 33us to 9us.
"""

import numpy as np
import ml_dtypes

import concourse.bass as bass
import concourse.mybir as mybir
import concourse.tile as tile
from concourse import bacc
from concourse.bass_utils import run_bass_kernel_spmd
from concourse.masks import make_identity

CUR, FULL, BS, DIM, H, D = 512, 1024, 4, 1024, 16, 64
NHC = 4                 # heads per core
NB = 2                  # batches per core
HDC = NHC * D           # 256 head-dims per core
SCALE = 1.0 / D ** 0.5  # 0.125
BIG = -30000.0
PADW = 1536             # padded row width for the shift round trip
RSTR = PADW - 1         # shifted read row stride
BF = mybir.dt.bfloat16
F32 = mybir.dt.float32
Exp = mybir.ActivationFunctionType.Exp
Copy = mybir.ActivationFunctionType.Copy
AluAdd = mybir.AluOpType.add

_CACHED = {}


def build_program():
    nc = bacc.Bacc(None, target_bir_lowering=False, debug=False)
    # One packed input tensor, phase-ordered so a handful of big DMAs
    # stream it in the order the projection loops consume it:
    #   R-block  8 k8-slices of (xpos 1024 | wpos 256)       = 8*1280
    #   Q-block  8 k8-slices of (xcur 2b*512 | wq 256)        = 8*1280
    #   K-block  8 k8-slices of (xfull 2b*1024 | wk 256)      = 8*2304
    #   V-block  8 k8-slices of (wv 256)                      = 8*256
    #   wproj    hd-major [128, 2, 1024]                      = 2048
    PCOLS = 8 * 1280 + 8 * 1280 + 8 * 2304 + 8 * 256 + 2048
    packed = nc.declare_dram_parameter("packed", [128, PCOLS], BF,
                                       isOutput=False)
    uvp_in = nc.declare_dram_parameter("uvp", [128, 4], F32, isOutput=False)
    outp = nc.declare_dram_parameter("outp", [CUR, NB * DIM], F32, isOutput=True)
    R0, Q0, K0, V0, W0 = (0, 8 * 1280, 8 * 2560, 8 * 2560 + 8 * 2304,
                          8 * 2560 + 8 * 2304 + 8 * 256)

    with tile.TileContext(nc) as tc:
        const = tc.alloc_tile_pool(name="const", bufs=1)
        psA = tc.alloc_tile_pool(name="psA", bufs=3, space="PSUM")
        psB = tc.alloc_tile_pool(name="psB", bufs=2, space="PSUM")
        psT = tc.alloc_tile_pool(name="psT", bufs=2, space="PSUM")
        psO = tc.alloc_tile_pool(name="psO", bufs=1, space="PSUM")
        slabp = tc.alloc_tile_pool(name="slabp", bufs=3)
        sp = tc.alloc_tile_pool(name="sp", bufs=3)
        atp = tc.alloc_tile_pool(name="atp", bufs=3)
        work = tc.alloc_tile_pool(name="work", bufs=2)
        dram = tc.alloc_tile_pool(name="dram", bufs=4, space="DRAM")

        # ---- resident SBUF tensors ----
        # pk_sb mirrors the packed DRAM layout 1:1
        pk_sb = const.tile([128, PCOLS], BF)
        uvp_sb = const.tile([128, 4], F32)
        ident = const.tile([128, 128], BF)
        big_sb = const.tile([128, 512], BF)
        kt_sb = const.tile([128, NB, 2, FULL], BF)   # K^T (dc, b, t, j)
        rt_sb = const.tile([128, 2, FULL], BF)       # R^T (dc, t, m)
        v_sb = const.tile([128, 8, NB, NHC, D + 1], BF)
        qt_sb = const.tile([128, NB, 2, 2, CUR], BF)  # Q^T +u/+v (dc,b,t,uv,i)
        o_sb = const.tile([128, NB, 2, CUR], BF)      # O^T normalized

        # packed views (k8-sliced)
        def xpos_v(k8):
            return pk_sb[:, R0 + k8 * 1280: R0 + k8 * 1280 + 1024]

        def wpos_v(k8, t):
            c = R0 + k8 * 1280 + 1024 + t * 128
            return pk_sb[:, c:c + 128]

        def xcur_v(k8, b):
            c = Q0 + k8 * 1280 + b * 512
            return pk_sb[:, c:c + 512]

        def wq_v(k8, t):
            c = Q0 + k8 * 1280 + 1024 + t * 128
            return pk_sb[:, c:c + 128]

        def xfull_v(k8, b, c0, c1):
            c = K0 + k8 * 2304 + b * 1024
            return pk_sb[:, c + c0:c + c1]

        def wk_v(k8, t):
            c = K0 + k8 * 2304 + 2048 + t * 128
            return pk_sb[:, c:c + 128]

        def wv_v(k8):
            c = V0 + k8 * 256
            return pk_sb[:, c:c + 256]

        def wproj_v(t, c0, c1):
            c = W0 + t * 1024
            return pk_sb[:, c + c0:c + c1]

        # phase-ordered streaming loads: few, large DMAs
        nc.sync.dma_start(out=uvp_sb[:], in_=uvp_in[:])
        bounds = [R0, R0 + 1280, R0 + 2 * 1280, R0 + 4 * 1280, R0 + 6 * 1280,
                  Q0, Q0 + 4 * 1280,
                  K0, K0 + 2304, K0 + 2 * 2304, K0 + 3 * 2304, K0 + 4 * 2304,
                  K0 + 5 * 2304, K0 + 6 * 2304, K0 + 7 * 2304,
                  V0, PCOLS]
        for c0, c1 in zip(bounds[:-1], bounds[1:]):
            nc.sync.dma_start(out=pk_sb[:, c0:c1], in_=packed[:, c0:c1])
        make_identity(nc, ident[:])
        nc.vector.memset(big_sb[:], BIG)
        nc.gpsimd.memset(v_sb[:, :, :, :, D:D + 1], 1.0)

        # shift buffers: one per unit slot (double-buffered across units);
        # pad columns [1024,1536) hold BIG once (causal mask for free).
        pdram = []
        for rep in range(4):
            t = dram.tile([CUR * PADW], BF, tag=f"pd_{rep}", name=f"pd_{rep}")
            pdram.append(t)
            for rq in range(4):
                nc.sync.dma_start(
                    out=bass.AP(tensor=t.tensor,
                                offset=rq * 128 * PADW + FULL,
                                ap=[[PADW, 128], [1, 512]]),
                    in_=big_sb[:])

        # ---- projections ----
        def stage_r_start():
            """All 4 R^T groups k8-major across 4 PSUM tiles, chasing the
            per-k8 input DMA slices."""
            tiles = [psA.tile([128, 512], F32, tag="pj", name=f"p_r0_{g}")
                     for g in range(2)]
            tiles += [psB.tile([128, 512], F32, tag="pv", name=f"p_r1_{g}")
                      for g in range(2)]
            for k8 in range(8):
                for g, pk in enumerate(tiles):
                    t, nh = g // 2, g % 2
                    nc.tensor.matmul(
                        pk[:], wpos_v(k8, t),
                        xpos_v(k8)[:, nh * 512:(nh + 1) * 512],
                        start=(k8 == 0), stop=(k8 == 7))
            for g, pk in enumerate(tiles):
                t, nh = g // 2, g % 2
                nc.vector.tensor_copy(
                    rt_sb[:, t, nh * 512:(nh + 1) * 512], pk[:])

        def stage_q():
            """Q k8-major across 4 PSUM tiles, chasing the xcur slices."""
            tiles = [psA.tile([128, 512], F32, tag="pj", name=f"p_q_{g}")
                     for g in range(2)]
            tiles += [psB.tile([128, 512], F32, tag="pv", name=f"p_q1_{g}")
                      for g in range(2)]
            for k8 in range(8):
                for g, pq in enumerate(tiles):
                    b, t = g // 2, g % 2
                    nc.tensor.matmul(
                        pq[:], wq_v(k8, t), xcur_v(k8, b),
                        start=(k8 == 0), stop=(k8 == 7))
            for g, pq in enumerate(tiles):
                b, t = g // 2, g % 2
                nc.vector.tensor_scalar_add(
                    qt_sb[:, b, t, 0, :], pq[:], uvp_sb[:, 2 * t:2 * t + 1])
                nc.vector.tensor_scalar_add(
                    qt_sb[:, b, t, 1, :], pq[:],
                    uvp_sb[:, 2 * t + 1:2 * t + 2])

        def pos_unit(u):
            for qt in range(4):
                a_pos_qt(u, qt)

        def stage_k(b):
            """K groups sequential (xfull slices stream in during R/Q)."""
            if True:
                for t in range(2):
                    for nh in range(2):
                        pk = psA.tile([128, 512], F32, tag="pj",
                                      name=f"p_k_{b}{t}{nh}")
                        for k8 in range(8):
                            nc.tensor.matmul(
                                pk[:], wk_v(k8, t),
                                xfull_v(k8, b, nh * 512, (nh + 1) * 512),
                                start=(k8 == 0), stop=(k8 == 7))
                        nc.vector.tensor_copy(
                            kt_sb[:, b, t, nh * 512:(nh + 1) * 512], pk[:])

        def v_group(b, jt):
            pv = psB.tile([128, HDC], F32, tag="pv", name=f"p_v_{b}{jt}")
            for k8 in range(8):
                nc.tensor.matmul(
                    pv[:], xfull_v(k8, b, jt * 128, (jt + 1) * 128),
                    wv_v(k8),
                    start=(k8 == 0), stop=(k8 == 7))
            nc.vector.tensor_copy(
                v_sb[:, jt, b, :, 0:D],
                pv[:].rearrange("p (h d) -> p h d", h=NHC))

        # ---- attention stages (per unit u = (h, b)) ----
        s_tiles = {}    # u -> content+shifted-pos scores [128, 4, FULL]
        at_tiles = {}   # u -> [128, 8, CUR] A^T blocks
        slab_tiles = {}
        ov_tiles = {}

        def hb(u):
            return u % 4, u // 4

        def evict(engine, out, in_):
            if engine == "v":
                nc.vector.tensor_copy(out, in_)
            else:
                nc.scalar.activation(out, in_, Copy)

        SLAB_ENG = {0: "v", 1: "a", 2: "v", 3: "a"}
        CONT_ENG = {(0, 0): "a", (1, 0): "v", (2, 0): "a", (3, 0): "a",
                    (0, 1): "v", (1, 1): "a", (2, 1): "v", (3, 1): "v"}

        def a_pos_qt(u, qt):
            """Position scores for one query tile: matmul, evict, write."""
            h, b = hb(u)
            p0 = (h % 2) * 64
            th = h // 2
            if qt == 0:
                slab_tiles[u] = slabp.tile([128, 4, FULL], BF, tag="slab",
                                           name=f"slab_{u}")
            slab = slab_tiles[u]
            m_min = 384 - 128 * qt
            c = m_min
            ci = 0
            while c < 1024:
                ce = min(c + 512, 1024)
                pp = psA.tile([128, ce - c], F32, tag="pj",
                              name=f"pp_{u}_{qt}_{ci}")
                nc.tensor.matmul(
                    pp[:],
                    qt_sb[p0:p0 + 64, b, th, 1, qt * 128:(qt + 1) * 128],
                    rt_sb[p0:p0 + 64, th, c:ce],
                    start=True, stop=True)
                evict(SLAB_ENG[qt], slab[:, qt, c:ce], pp[:])
                c = ce
                ci += 1
            pd = pdram[u % 4]
            nc.sync.dma_start(
                out=bass.AP(tensor=pd.tensor,
                            offset=qt * 128 * PADW + m_min,
                            ap=[[PADW, 128], [1, 1024 - m_min]]),
                in_=slab[:, qt, m_min:1024])

        def a_content_qt(u, qt):
            """Content scores for one tile + shifted-pos accumulate DMA."""
            h, b = hb(u)
            p0 = (h % 2) * 64
            th = h // 2
            pd = pdram[u % 4]
            if qt == 0:
                s_tiles[u] = sp.tile([128, 4, FULL], BF, tag="s", name=f"s_{u}")
            s_all = s_tiles[u]
            jw = 640 + 128 * qt
            c = 0
            ci = 0
            while c < jw:
                ce = min(c + 512, jw)
                pc = psB.tile([128, ce - c], F32, tag="pv",
                              name=f"pc_{u}_{qt}_{ci}")
                nc.tensor.matmul(
                    pc[:],
                    qt_sb[p0:p0 + 64, b, th, 0, qt * 128:(qt + 1) * 128],
                    kt_sb[p0:p0 + 64, b, th, c:ce],
                    start=True, stop=True)
                evict(CONT_ENG[(qt, ci)], s_all[:, qt, c:ce], pc[:])
                c = ce
                ci += 1
            # shifted position rows accumulate onto the content scores
            nc.gpsimd.dma_start(
                out=s_all[:, qt, 0:jw],
                in_=bass.AP(tensor=pd.tensor,
                            offset=qt * 128 * PADW + 511 - 128 * qt,
                            ap=[[RSTR, 128], [1, jw]]),
                accum_op=AluAdd)

        def e_qt(u, qt, tpool=None):
            """Transpose hull blocks of one tile, exponentiate into A^T."""
            if qt == 0:
                at_tiles[u] = atp.tile([128, 8, CUR], BF, tag="at",
                                       name=f"at_{u}")
            at_all = at_tiles[u]
            s_all = s_tiles[u]
            nj8 = qt + 5
            st = (tpool or psT).tile([128, 8, 128], BF,
                                     tag="pj" if tpool else "pt",
                                     name=f"st_{u}_{qt}")
            for j8 in range(nj8):
                nc.tensor.transpose(st[:, j8, :],
                                    s_all[:, qt, j8 * 128:(j8 + 1) * 128],
                                    ident[:])
            nc.scalar.activation(
                at_all[:, 0:nj8, qt * 128:(qt + 1) * 128],
                st[:, 0:nj8, :], Exp, scale=SCALE)

        def f_av(u, jts, pool=None):
            """Part of A^T @ V accumulation (ones column -> denominator)."""
            h, b = hb(u)
            if jts[0] == 0:
                ov_tiles[u] = (pool or psO).tile(
                    [D + 1, CUR], F32, tag="pv" if pool else "po",
                    name=f"ov_{u}")
            ov = ov_tiles[u]
            at_all = at_tiles[u]
            for jt in jts:
                c0 = max(0, (jt - 4) * 128)
                nc.tensor.matmul(ov[:, c0:], v_sb[:, jt, b, h, :],
                                 at_all[:, jt, c0:],
                                 start=(jt == 0), stop=(jt == 7),
                                 skip_group_check=True)

        def f_norm(u, chunks=((0, CUR),), done=True):
            """Normalize by the softmax denominator into O^T."""
            h, b = hb(u)
            p0 = (h % 2) * 64
            th = h // 2
            ov = ov_tiles[u]
            if done:
                ov_tiles.pop(u)
                at_tiles.pop(u, None)
            for c0, c1 in chunks:
                rden = work.tile([1, CUR], F32, tag="rden", bufs=2,
                                 name=f"rden_{u}_{c0}")
                nc.vector.reciprocal(rden[0:1, 0:c1 - c0], ov[D:D + 1, c0:c1])
                rdb = work.tile([64, CUR], F32, tag="rdb", bufs=2,
                                name=f"rdb_{u}_{c0}")
                nc.gpsimd.partition_broadcast(rdb[0:64, 0:c1 - c0],
                                              rden[0:1, 0:c1 - c0])
                nc.vector.tensor_mul(o_sb[p0:p0 + 64, b, th, c0:c1],
                                     ov[0:D, c0:c1], rdb[0:64, 0:c1 - c0])

        def unit_slot(up, uc, ue, vgs=(), projqts=()):
            """One pipeline slot: position scores for unit up, content
            scores for unit uc, transpose/exp/AV for unit ue, plus
            V-projection or output-projection fillers."""
            vit = iter(vgs)
            pit = iter(projqts)
            for qt in range(4):
                if ue is not None:
                    e_qt(ue, qt)
                v = next(vit, None)
                if v is not None:
                    v_group(*v)
                if up is not None:
                    a_pos_qt(up, qt)
                if qt % 2 == 1:
                    p = next(pit, None)
                    if p is not None:
                        proj_qt(*p, pools=((psA, "pj"), (psA, "pj")))
            for qt in range(4):
                v = next(vit, None)
                if v is not None:
                    v_group(*v)
                if uc is not None:
                    a_content_qt(uc, qt)
                if ue is not None and qt % 2 == 1:
                    f_av(ue, ((qt - 1) * 2, (qt - 1) * 2 + 1,
                              (qt - 1) * 2 + 2, (qt - 1) * 2 + 3))
            if ue is not None:
                f_norm(ue)

        # ---- output projection ----
        proj_tiles = {}

        def proj_start(b, qt, pools=None, nhs=(0, 1)):
            for nh in nhs:
                pool, tg = (pools or ((psB, "pv"), (psO, "po")))[nh]
                pr = pool.tile([128, 512], F32, tag=tg, name=f"pr_{b}{qt}{nh}")
                proj_tiles[(b, qt, nh)] = pr
                nc.tensor.matmul(
                    pr[:], o_sb[:, b, 0, qt * 128:(qt + 1) * 128],
                    wproj_v(0, nh * 512, (nh + 1) * 512),
                    start=True, stop=False)

        def proj_finish(b, qt):
            ot = work.tile([128, FULL], F32, tag="ot", bufs=2,
                           name=f"ot_{b}{qt}")
            for nh in range(2):
                if (b, qt, nh) not in proj_tiles:
                    proj_start(b, qt, nhs=(nh,))
                pr = proj_tiles.pop((b, qt, nh))
                nc.tensor.matmul(
                    pr[:], o_sb[:, b, 1, qt * 128:(qt + 1) * 128],
                    wproj_v(1, nh * 512, (nh + 1) * 512),
                    start=False, stop=True)
                evict("a" if nh == 0 else "v",
                      ot[:, nh * 512:(nh + 1) * 512], pr[:])
                nc.sync.dma_start(
                    out=outp[qt * 128:(qt + 1) * 128,
                             b * DIM + nh * 512:b * DIM + (nh + 1) * 512],
                    in_=ot[:, nh * 512:(nh + 1) * 512])

        def proj_qt(b, qt, pools=None):
            proj_start(b, qt, pools=pools)
            proj_finish(b, qt)

        # ---- schedule ----
        # 3-phase software pipeline (pos / content / ef decoupled).
        # pos(0..3) run right after R+Q, inside the DMA-bound startup
        # window, before the xfull-dependent K/V matmuls enter the PE
        # queue; content follows per-batch K; ef trails by 2 slots.
        vlist = [(b, jt) for b in range(NB) for jt in range(8)]
        stage_r_start()
        stage_q()
        for u in range(4):
            pos_unit(u)
        stage_k(0)
        unit_slot(None, 0, None, vgs=vlist[0:4])
        stage_k(1)
        unit_slot(None, 1, None, vgs=vlist[4:8])
        unit_slot(4, 2, 0, vgs=vlist[8:12])
        unit_slot(5, 3, 1, vgs=vlist[12:16])
        unit_slot(6, 4, 2)
        unit_slot(7, 5, 3)
        unit_slot(None, 6, 4)
        unit_slot(None, 7, 5, projqts=((0, 0), (0, 1)))
        unit_slot(None, None, 6, projqts=((0, 2), (0, 3)))
        # tail: last unit (batch 1) alone, projection interleaved
        u7 = 7
        e_qt(u7, 0)
        e_qt(u7, 1)
        e_qt(u7, 2)
        e_qt(u7, 3)
        f_av(u7, (0, 1, 2, 3))
        proj_start(1, 0, pools=((psA, "pj"), (psA, "pj")))
        f_av(u7, (4, 5, 6, 7))

        for qt in range(4):
            f_norm(u7, chunks=((qt * 128, (qt + 1) * 128),), done=(qt == 3))
            if qt == 0:
                proj_finish(1, 0)
            else:
                proj_qt(1, qt)

        for p in (dram, work, atp, sp, slabp, psO, psT, psB, psA, const):
            p.release()
    nc.compile()
    return nc


def kernel(inputs, pos_embedding, full_input, u, v, W_kv, b_kv, W_q, b_q,
           W_pos, b_pos, W_proj, b_proj, mask):
    bf = ml_dtypes.bfloat16
    inputs = np.asarray(inputs)
    full_input = np.asarray(full_input)
    pos = np.asarray(pos_embedding)[:, 0, :]

    if "nc" not in _CACHED:
        _CACHED["nc"] = build_program()
    nc = _CACHED["nc"]

    PCOLS = 8 * 1280 + 8 * 1280 + 8 * 2304 + 8 * 256 + 2048
    R0, Q0, K0, V0, W0 = (0, 8 * 1280, 8 * 2560, 8 * 2560 + 8 * 2304,
                          8 * 2560 + 8 * 2304 + 8 * 256)
    posT = pos.T.astype(bf)                      # [DIM, FULL]
    in_maps = []
    for c in range(8):
        bg, hg = c // 4, c % 4
        sl = slice(hg * HDC, (hg + 1) * HDC)
        bsl = slice(2 * bg, 2 * bg + 2)
        uvec = (np.asarray(u).reshape(-1) + np.asarray(b_q))[sl]
        vvec = (np.asarray(v).reshape(-1) + np.asarray(b_q))[sl]
        # uvp[p, 2t+0/1] = u/v for head-dim t*128+p
        uvp = np.stack([uvec[0:128], vvec[0:128],
                        uvec[128:256], vvec[128:256]], axis=1)
        xf = full_input[:, bsl, :].transpose(2, 1, 0).astype(bf)  # [DIM,2,FULL]
        xc = inputs[:, bsl, :].transpose(2, 1, 0).astype(bf)      # [DIM,2,CUR]
        wq = W_q[:, sl].astype(bf)
        wk = W_kv[:, hg * HDC:(hg + 1) * HDC].astype(bf)
        wv = W_kv[:, H * D + hg * HDC:H * D + (hg + 1) * HDC].astype(bf)
        wpos = W_pos[:, sl].astype(bf)
        wproj = W_proj[sl, :].astype(bf)          # [HDC, DIM]

        P = np.zeros((128, PCOLS), bf)
        for k8 in range(8):
            dsl = slice(k8 * 128, (k8 + 1) * 128)
            P[:, R0 + k8 * 1280:R0 + k8 * 1280 + 1024] = posT[dsl]
            P[:, R0 + k8 * 1280 + 1024:R0 + (k8 + 1) * 1280] = wpos[dsl]
            P[:, Q0 + k8 * 1280:Q0 + k8 * 1280 + 1024] = \
                xc[dsl].reshape(128, NB * CUR)
            P[:, Q0 + k8 * 1280 + 1024:Q0 + (k8 + 1) * 1280] = wq[dsl]
            P[:, K0 + k8 * 2304:K0 + k8 * 2304 + 2048] = \
                xf[dsl].reshape(128, NB * FULL)
            P[:, K0 + k8 * 2304 + 2048:K0 + (k8 + 1) * 2304] = wk[dsl]
            P[:, V0 + k8 * 256:V0 + (k8 + 1) * 256] = wv[dsl]
        P[:, W0:W0 + 1024] = wproj[0:128]
        P[:, W0 + 1024:W0 + 2048] = wproj[128:256]
        in_maps.append({
            "packed": P,
            "uvp": np.ascontiguousarray(uvp).astype(np.float32),
        })

    _CACHED["maps"] = in_maps
    res = run_bass_kernel_spmd(nc, in_maps, list(range(8)))
    out = np.zeros((CUR, BS, DIM), np.float32)
    for c in range(8):
        bg, hg = c // 4, c % 4
        r = res.results[c]["outp"].reshape(CUR, NB, DIM)
        out[:, 2 * bg, :] += r[:, 0, :]
        out[:, 2 * bg + 1, :] += r[:, 1, :]
    return out


# revision 19
# speedup vs baseline: 1.0844x; 1.0838x over previous
"""TransformerXL relative attention on 8 TRN2 NeuronCores — v3.

Sharding: TP over heads 4-way x DP over batch 2-way.  Core c handles
batch group bg=c//4 (batches 2bg, 2bg+1) and head group hg=c%4 (4 heads,
256 head-dims).  Each core computes a partial output projection
[CUR, 2, DIM]; the host sums the 4 head-group partials per batch.

v3 vs v2 (129.3us):
- TP4xDP2 instead of TP2xDP4: the batch-independent R^T = pos @ W_pos
  projection halves (2 t-tiles instead of 4), saving 16384 PE cycles;
  everything else is work-neutral (8 (head,batch) attention units per
  core either way).
- The +u / +v query biases are folded into the Q eviction as DVE
  tensor_scalar adds (per-partition scalar AP), removing the PE
  ones-row matmuls (-4096 cycles).
- The shifted position-score readback is one 3-D AP accum DMA per unit
  ([[RSTR,128],[128*RSTR,4],[1,1024]]) instead of 4 per-qt reads,
  cutting SWDGE descriptor-gen on GPSIMD from 33us to 9us.
"""

import numpy as np
import ml_dtypes

import concourse.bass as bass
import concourse.mybir as mybir
import concourse.tile as tile
from concourse import bacc
from concourse.bass_utils import run_bass_kernel_spmd
from concourse.masks import make_identity

CUR, FULL, BS, DIM, H, D = 512, 1024, 4, 1024, 16, 64
NHC = 4                 # heads per core
NB = 2                  # batches per core
HDC = NHC * D           # 256 head-dims per core
SCALE = 1.0 / D ** 0.5  # 0.125
BIG = -30000.0
PADW = 1536             # padded row width for the shift round trip
RSTR = PADW - 1         # shifted read row stride
BF = mybir.dt.bfloat16
F32 = mybir.dt.float32
Exp = mybir.ActivationFunctionType.Exp
Copy = mybir.ActivationFunctionType.Copy
AluAdd = mybir.AluOpType.add

_CACHED = {}


def build_program():
    nc = bacc.Bacc(None, target_bir_lowering=False, debug=False)
    # One packed input tensor, phase-ordered so a handful of big DMAs
    # stream it in the order the projection loops consume it:
    #   R-block  8 k8-slices of (xpos 1024 | wpos 256)       = 8*1280
    #   Q-block  8 k8-slices of (xcur 2b*512 | wq 256)        = 8*1280
    #   K-block  8 k8-slices of (xfull 2b*1024 | wk 256)      = 8*2304
    #   V-block  8 k8-slices of (wv 256)                      = 8*256
    #   wproj    hd-major [128, 2, 1024]                      = 2048
    PCOLS = 8 * 1280 + 8 * 1280 + 8 * 2304 + 8 * 256 + 2048
    packed = nc.declare_dram_parameter("packed", [128, PCOLS], BF,
                                       isOutput=False)
    uvp_in = nc.declare_dram_parameter("uvp", [128, 4], F32, isOutput=False)
    outp = nc.declare_dram_parameter("outp", [CUR, NB * DIM], F32, isOutput=True)
    R0, Q0, K0, V0, W0 = (0, 8 * 1280, 8 * 2560, 8 * 2560 + 8 * 2304,
                          8 * 2560 + 8 * 2304 + 8 * 256)

    with tile.TileContext(nc) as tc:
        const = tc.alloc_tile_pool(name="const", bufs=1)
        psA = tc.alloc_tile_pool(name="psA", bufs=3, space="PSUM")
        psB = tc.alloc_tile_pool(name="psB", bufs=2, space="PSUM")
        psT = tc.alloc_tile_pool(name="psT", bufs=2, space="PSUM")
        psO = tc.alloc_tile_pool(name="psO", bufs=1, space="PSUM")
        slabp = tc.alloc_tile_pool(name="slabp", bufs=3)
        sp = tc.alloc_tile_pool(name="sp", bufs=3)
        atp = tc.alloc_tile_pool(name="atp", bufs=3)
        work = tc.alloc_tile_pool(name="work", bufs=2)
        dram = tc.alloc_tile_pool(name="dram", bufs=4, space="DRAM")

        # ---- resident SBUF tensors ----
        # pk_sb mirrors the packed DRAM layout 1:1
        pk_sb = const.tile([128, PCOLS], BF)
        uvp_sb = const.tile([128, 4], F32)
        ident = const.tile([128, 128], BF)
        big_sb = const.tile([128, 512], BF)
        kt_sb = const.tile([128, NB, 2, FULL], BF)   # K^T (dc, b, t, j)
        rt_sb = const.tile([128, 2, FULL], BF)       # R^T (dc, t, m)
        v_sb = const.tile([128, 8, NB, NHC, D + 1], BF)
        qt_sb = const.tile([128, NB, 2, 2, CUR], BF)  # Q^T +u/+v (dc,b,t,uv,i)
        o_sb = const.tile([128, NB, 2, CUR], BF)      # O^T normalized

        # packed views (k8-sliced)
        def xpos_v(k8):
            return pk_sb[:, R0 + k8 * 1280: R0 + k8 * 1280 + 1024]

        def wpos_v(k8, t):
            c = R0 + k8 * 1280 + 1024 + t * 128
            return pk_sb[:, c:c + 128]

        def xcur_v(k8, b):
            c = Q0 + k8 * 1280 + b * 512
            return pk_sb[:, c:c + 512]

        def wq_v(k8, t):
            c = Q0 + k8 * 1280 + 1024 + t * 128
            return pk_sb[:, c:c + 128]

        def xfull_v(k8, b, c0, c1):
            c = K0 + k8 * 2304 + b * 1024
            return pk_sb[:, c + c0:c + c1]

        def wk_v(k8, t):
            c = K0 + k8 * 2304 + 2048 + t * 128
            return pk_sb[:, c:c + 128]

        def wv_v(k8):
            c = V0 + k8 * 256
            return pk_sb[:, c:c + 256]

        def wproj_v(t, c0, c1):
            c = W0 + t * 1024
            return pk_sb[:, c + c0:c + c1]

        # phase-ordered streaming loads: few, large DMAs
        nc.sync.dma_start(out=uvp_sb[:], in_=uvp_in[:])
        bounds = [R0, R0 + 1280, R0 + 2 * 1280, R0 + 4 * 1280, R0 + 6 * 1280,
                  Q0, Q0 + 4 * 1280,
                  K0, K0 + 2304, K0 + 2 * 2304, K0 + 3 * 2304, K0 + 4 * 2304,
                  K0 + 5 * 2304, K0 + 6 * 2304, K0 + 7 * 2304,
                  V0, PCOLS]
        for c0, c1 in zip(bounds[:-1], bounds[1:]):
            nc.sync.dma_start(out=pk_sb[:, c0:c1], in_=packed[:, c0:c1])
        make_identity(nc, ident[:])
        nc.vector.memset(big_sb[:], BIG)
        nc.gpsimd.memset(v_sb[:, :, :, :, D:D + 1], 1.0)

        # shift buffers: one per unit slot (double-buffered across units);
        # pad columns [1024,1536) hold BIG once (causal mask for free).
        pdram = []
        for rep in range(4):
            t = dram.tile([CUR * PADW], BF, tag=f"pd_{rep}", name=f"pd_{rep}")
            pdram.append(t)
            for rq in range(4):
                nc.sync.dma_start(
                    out=bass.AP(tensor=t.tensor,
                                offset=rq * 128 * PADW + FULL,
                                ap=[[PADW, 128], [1, 512]]),
                    in_=big_sb[:])

        # ---- projections ----
        def stage_r_start():
            """All 4 R^T groups k8-major across 4 PSUM tiles, chasing the
            per-k8 input DMA slices."""
            tiles = [psA.tile([128, 512], F32, tag="pj", name=f"p_r0_{g}")
                     for g in range(2)]
            tiles += [psB.tile([128, 512], F32, tag="pv", name=f"p_r1_{g}")
                      for g in range(2)]
            for k8 in range(8):
                for g, pk in enumerate(tiles):
                    t, nh = g // 2, g % 2
                    nc.tensor.matmul(
                        pk[:], wpos_v(k8, t),
                        xpos_v(k8)[:, nh * 512:(nh + 1) * 512],
                        start=(k8 == 0), stop=(k8 == 7))
            for g, pk in enumerate(tiles):
                t, nh = g // 2, g % 2
                nc.vector.tensor_copy(
                    rt_sb[:, t, nh * 512:(nh + 1) * 512], pk[:])

        def stage_q():
            """Q k8-major across 4 PSUM tiles, chasing the xcur slices."""
            tiles = [psA.tile([128, 512], F32, tag="pj", name=f"p_q_{g}")
                     for g in range(2)]
            tiles += [psB.tile([128, 512], F32, tag="pv", name=f"p_q1_{g}")
                      for g in range(2)]
            for k8 in range(8):
                for g, pq in enumerate(tiles):
                    b, t = g // 2, g % 2
                    nc.tensor.matmul(
                        pq[:], wq_v(k8, t), xcur_v(k8, b),
                        start=(k8 == 0), stop=(k8 == 7))
            for g, pq in enumerate(tiles):
                b, t = g // 2, g % 2
                nc.vector.tensor_scalar_add(
                    qt_sb[:, b, t, 0, :], pq[:], uvp_sb[:, 2 * t:2 * t + 1])
                nc.vector.tensor_scalar_add(
                    qt_sb[:, b, t, 1, :], pq[:],
                    uvp_sb[:, 2 * t + 1:2 * t + 2])

        def pos_unit(u):
            for qt in range(4):
                a_pos_qt(u, qt)

        def stage_k(b):
            """K groups sequential (xfull slices stream in during R/Q)."""
            if True:
                for t in range(2):
                    for nh in range(2):
                        pk = psA.tile([128, 512], F32, tag="pj",
                                      name=f"p_k_{b}{t}{nh}")
                        for k8 in range(8):
                            nc.tensor.matmul(
                                pk[:], wk_v(k8, t),
                                xfull_v(k8, b, nh * 512, (nh + 1) * 512),
                                start=(k8 == 0), stop=(k8 == 7))
                        nc.vector.tensor_copy(
                            kt_sb[:, b, t, nh * 512:(nh + 1) * 512], pk[:])

        def v_group(b, jt):
            pv = psB.tile([128, HDC], F32, tag="pv", name=f"p_v_{b}{jt}")
            for k8 in range(8):
                nc.tensor.matmul(
                    pv[:], xfull_v(k8, b, jt * 128, (jt + 1) * 128),
                    wv_v(k8),
                    start=(k8 == 0), stop=(k8 == 7))
            nc.vector.tensor_copy(
                v_sb[:, jt, b, :, 0:D],
                pv[:].rearrange("p (h d) -> p h d", h=NHC))

        # ---- attention stages (per unit u = (h, b)) ----
        s_tiles = {}    # u -> content+shifted-pos scores [128, 4, FULL]
        at_tiles = {}   # u -> [128, 8, CUR] A^T blocks
        slab_tiles = {}
        ov_tiles = {}

        def hb(u):
            return u % 4, u // 4

        def evict(engine, out, in_):
            if engine == "v":
                nc.vector.tensor_copy(out, in_)
            else:
                nc.scalar.activation(out, in_, Copy)

        SLAB_ENG = {qt: "v" for qt in range(4)}
        CONT_ENG = {(0, 0): "a", (1, 0): "a", (2, 0): "a", (3, 0): "a",
                    (0, 1): "a", (1, 1): "a", (2, 1): "v", (3, 1): "v"}

        def a_pos_qt(u, qt):
            """Position scores for one query tile: matmul, evict, write."""
            h, b = hb(u)
            p0 = (h % 2) * 64
            th = h // 2
            if qt == 0:
                slab_tiles[u] = slabp.tile([128, 4, FULL], BF, tag="slab",
                                           name=f"slab_{u}")
            slab = slab_tiles[u]
            m_min = 384 - 128 * qt
            c = m_min
            ci = 0
            while c < 1024:
                ce = min(c + 512, 1024)
                pp = psA.tile([128, ce - c], F32, tag="pj",
                              name=f"pp_{u}_{qt}_{ci}")
                nc.tensor.matmul(
                    pp[:],
                    qt_sb[p0:p0 + 64, b, th, 1, qt * 128:(qt + 1) * 128],
                    rt_sb[p0:p0 + 64, th, c:ce],
                    start=True, stop=True)
                evict(SLAB_ENG[qt], slab[:, qt, c:ce], pp[:])
                c = ce
                ci += 1
            pd = pdram[u % 4]
            nc.sync.dma_start(
                out=bass.AP(tensor=pd.tensor,
                            offset=qt * 128 * PADW + m_min,
                            ap=[[PADW, 128], [1, 1024 - m_min]]),
                in_=slab[:, qt, m_min:1024])

        def a_content_qt(u, qt):
            """Content scores for one tile + shifted-pos accumulate DMA."""
            h, b = hb(u)
            p0 = (h % 2) * 64
            th = h // 2
            pd = pdram[u % 4]
            if qt == 0:
                s_tiles[u] = sp.tile([128, 4, FULL], BF, tag="s", name=f"s_{u}")
            s_all = s_tiles[u]
            jw = 640 + 128 * qt
            c = 0
            ci = 0
            while c < jw:
                ce = min(c + 512, jw)
                pc = psB.tile([128, ce - c], F32, tag="pv",
                              name=f"pc_{u}_{qt}_{ci}")
                nc.tensor.matmul(
                    pc[:],
                    qt_sb[p0:p0 + 64, b, th, 0, qt * 128:(qt + 1) * 128],
                    kt_sb[p0:p0 + 64, b, th, c:ce],
                    start=True, stop=True)
                evict(CONT_ENG[(qt, ci)], s_all[:, qt, c:ce], pc[:])
                c = ce
                ci += 1
            # shifted position rows accumulate onto the content scores
            nc.gpsimd.dma_start(
                out=s_all[:, qt, 0:jw],
                in_=bass.AP(tensor=pd.tensor,
                            offset=qt * 128 * PADW + 511 - 128 * qt,
                            ap=[[RSTR, 128], [1, jw]]),
                accum_op=AluAdd)

        def e_qt(u, qt, tpool=None):
            """Transpose hull blocks of one tile, exponentiate into A^T."""
            if qt == 0:
                at_tiles[u] = atp.tile([128, 8, CUR], BF, tag="at",
                                       name=f"at_{u}")
            at_all = at_tiles[u]
            s_all = s_tiles[u]
            nj8 = qt + 5
            st = (tpool or psT).tile([128, 8, 128], BF,
                                     tag="pj" if tpool else "pt",
                                     name=f"st_{u}_{qt}")
            for j8 in range(nj8):
                nc.tensor.transpose(st[:, j8, :],
                                    s_all[:, qt, j8 * 128:(j8 + 1) * 128],
                                    ident[:])
            nc.scalar.activation(
                at_all[:, 0:nj8, qt * 128:(qt + 1) * 128],
                st[:, 0:nj8, :], Exp, scale=SCALE)

        def f_av(u, jts, pool=None):
            """Part of A^T @ V accumulation (ones column -> denominator)."""
            h, b = hb(u)
            if jts[0] == 0:
                ov_tiles[u] = (pool or psO).tile(
                    [D + 1, CUR], F32, tag="pv" if pool else "po",
                    name=f"ov_{u}")
            ov = ov_tiles[u]
            at_all = at_tiles[u]
            for jt in jts:
                c0 = max(0, (jt - 4) * 128)
                nc.tensor.matmul(ov[:, c0:], v_sb[:, jt, b, h, :],
                                 at_all[:, jt, c0:],
                                 start=(jt == 0), stop=(jt == 7),
                                 skip_group_check=True)

        def f_norm(u, chunks=((0, CUR),), done=True):
            """Normalize by the softmax denominator into O^T."""
            h, b = hb(u)
            p0 = (h % 2) * 64
            th = h // 2
            ov = ov_tiles[u]
            if done:
                ov_tiles.pop(u)
                at_tiles.pop(u, None)
            for c0, c1 in chunks:
                rden = work.tile([1, CUR], F32, tag="rden", bufs=2,
                                 name=f"rden_{u}_{c0}")
                nc.vector.reciprocal(rden[0:1, 0:c1 - c0], ov[D:D + 1, c0:c1])
                rdb = work.tile([64, CUR], F32, tag="rdb", bufs=2,
                                name=f"rdb_{u}_{c0}")
                nc.gpsimd.partition_broadcast(rdb[0:64, 0:c1 - c0],
                                              rden[0:1, 0:c1 - c0])
                nc.vector.tensor_mul(o_sb[p0:p0 + 64, b, th, c0:c1],
                                     ov[0:D, c0:c1], rdb[0:64, 0:c1 - c0])

        def unit_slot(up, uc, ue, vgs=(), projqts=()):
            """One pipeline slot: position scores for unit up, content
            scores for unit uc, transpose/exp/AV for unit ue, plus
            V-projection or output-projection fillers."""
            vit = iter(vgs)
            pit = iter(projqts)
            for qt in range(4):
                if ue is not None:
                    e_qt(ue, qt)
                if up is not None:
                    a_pos_qt(up, qt)
                v = next(vit, None)
                if v is not None:
                    v_group(*v)
                if qt % 2 == 1:
                    p = next(pit, None)
                    if p is not None:
                        proj_qt(*p, pools=((psA, "pj"), (psA, "pj")))
            for qt in range(4):
                v = next(vit, None)
                if v is not None:
                    v_group(*v)
                if uc is not None:
                    a_content_qt(uc, qt)
                if ue is not None and qt % 2 == 1:
                    f_av(ue, ((qt - 1) * 2, (qt - 1) * 2 + 1,
                              (qt - 1) * 2 + 2, (qt - 1) * 2 + 3))
            if ue is not None:
                f_norm(ue)

        # ---- output projection ----
        proj_tiles = {}

        def proj_start(b, qt, pools=None, nhs=(0, 1)):
            for nh in nhs:
                pool, tg = (pools or ((psB, "pv"), (psO, "po")))[nh]
                pr = pool.tile([128, 512], F32, tag=tg, name=f"pr_{b}{qt}{nh}")
                proj_tiles[(b, qt, nh)] = pr
                nc.tensor.matmul(
                    pr[:], o_sb[:, b, 0, qt * 128:(qt + 1) * 128],
                    wproj_v(0, nh * 512, (nh + 1) * 512),
                    start=True, stop=False)

        def proj_finish(b, qt):
            ot = work.tile([128, FULL], F32, tag="ot", bufs=2,
                           name=f"ot_{b}{qt}")
            for nh in range(2):
                if (b, qt, nh) not in proj_tiles:
                    proj_start(b, qt, nhs=(nh,))
                pr = proj_tiles.pop((b, qt, nh))
                nc.tensor.matmul(
                    pr[:], o_sb[:, b, 1, qt * 128:(qt + 1) * 128],
                    wproj_v(1, nh * 512, (nh + 1) * 512),
                    start=False, stop=True)
                evict("a" if nh == 0 else "v",
                      ot[:, nh * 512:(nh + 1) * 512], pr[:])
                nc.sync.dma_start(
                    out=outp[qt * 128:(qt + 1) * 128,
                             b * DIM + nh * 512:b * DIM + (nh + 1) * 512],
                    in_=ot[:, nh * 512:(nh + 1) * 512])

        def proj_qt(b, qt, pools=None):
            proj_start(b, qt, pools=pools)
            proj_finish(b, qt)

        # ---- schedule ----
        # 3-phase software pipeline (pos / content / ef decoupled).
        # pos(0..3) run right after R+Q, inside the DMA-bound startup
        # window, before the xfull-dependent K/V matmuls enter the PE
        # queue; content follows per-batch K; ef trails by 2 slots.
        vlist = [(b, jt) for b in range(NB) for jt in range(8)]
        stage_r_start()
        stage_q()
        stage_k(0)
        stage_k(1)
        unit_slot(0, 0, None, vgs=vlist[0:4])
        unit_slot(1, 1, None, vgs=vlist[4:12])
        unit_slot(2, 2, 0, vgs=vlist[12:16])
        for u in range(3, 6):
            unit_slot(u, u, u - 2)
        unit_slot(6, 6, 4, projqts=((0, 0), (0, 1)))
        unit_slot(7, 7, 5, projqts=((0, 2), (0, 3)))
        # tail: interleave the last two units (both batch 1)
        u6, u7 = 6, 7
        for qt in range(4):
            e_qt(u6, qt)
        e_qt(u7, 0)
        e_qt(u7, 1)
        f_av(u6, (0, 1, 2, 3))
        e_qt(u7, 2)
        f_av(u6, (4, 5, 6, 7))
        e_qt(u7, 3)
        proj_start(1, 0, pools=((psA, "pj"), (psA, "pj")))
        f_norm(u6)
        f_av(u7, (0, 1, 2, 3), pool=psB)
        f_av(u7, (4, 5, 6, 7), pool=psB)

        for qt in range(4):
            f_norm(u7, chunks=((qt * 128, (qt + 1) * 128),), done=(qt == 3))
            if qt == 0:
                proj_finish(1, 0)
            else:
                proj_qt(1, qt)

        for p in (dram, work, atp, sp, slabp, psO, psT, psB, psA, const):
            p.release()
    nc.compile()
    return nc


def kernel(inputs, pos_embedding, full_input, u, v, W_kv, b_kv, W_q, b_q,
           W_pos, b_pos, W_proj, b_proj, mask):
    bf = ml_dtypes.bfloat16
    inputs = np.asarray(inputs)
    full_input = np.asarray(full_input)
    pos = np.asarray(pos_embedding)[:, 0, :]

    if "nc" not in _CACHED:
        _CACHED["nc"] = build_program()
    nc = _CACHED["nc"]

    PCOLS = 8 * 1280 + 8 * 1280 + 8 * 2304 + 8 * 256 + 2048
    R0, Q0, K0, V0, W0 = (0, 8 * 1280, 8 * 2560, 8 * 2560 + 8 * 2304,
                          8 * 2560 + 8 * 2304 + 8 * 256)
    posT = pos.T.astype(bf)                      # [DIM, FULL]
    in_maps = []
    for c in range(8):
        bg, hg = c // 4, c % 4
        sl = slice(hg * HDC, (hg + 1) * HDC)
        bsl = slice(2 * bg, 2 * bg + 2)
        uvec = (np.asarray(u).reshape(-1) + np.asarray(b_q))[sl]
        vvec = (np.asarray(v).reshape(-1) + np.asarray(b_q))[sl]
        # uvp[p, 2t+0/1] = u/v for head-dim t*128+p
        uvp = np.stack([uvec[0:128], vvec[0:128],
                        uvec[128:256], vvec[128:256]], axis=1)
        xf = full_input[:, bsl, :].transpose(2, 1, 0).astype(bf)  # [DIM,2,FULL]
        xc = inputs[:, bsl, :].transpose(2, 1, 0).astype(bf)      # [DIM,2,CUR]
        wq = W_q[:, sl].astype(bf)
        wk = W_kv[:, hg * HDC:(hg + 1) * HDC].astype(bf)
        wv = W_kv[:, H * D + hg * HDC:H * D + (hg + 1) * HDC].astype(bf)
        wpos = W_pos[:, sl].astype(bf)
        wproj = W_proj[sl, :].astype(bf)          # [HDC, DIM]

        P = np.zeros((128, PCOLS), bf)
        for k8 in range(8):
            dsl = slice(k8 * 128, (k8 + 1) * 128)
            P[:, R0 + k8 * 1280:R0 + k8 * 1280 + 1024] = posT[dsl]
            P[:, R0 + k8 * 1280 + 1024:R0 + (k8 + 1) * 1280] = wpos[dsl]
            P[:, Q0 + k8 * 1280:Q0 + k8 * 1280 + 1024] = \
                xc[dsl].reshape(128, NB * CUR)
            P[:, Q0 + k8 * 1280 + 1024:Q0 + (k8 + 1) * 1280] = wq[dsl]
            P[:, K0 + k8 * 2304:K0 + k8 * 2304 + 2048] = \
                xf[dsl].reshape(128, NB * FULL)
            P[:, K0 + k8 * 2304 + 2048:K0 + (k8 + 1) * 2304] = wk[dsl]
            P[:, V0 + k8 * 256:V0 + (k8 + 1) * 256] = wv[dsl]
        P[:, W0:W0 + 1024] = wproj[0:128]
        P[:, W0 + 1024:W0 + 2048] = wproj[128:256]
        in_maps.append({
            "packed": P,
            "uvp": np.ascontiguousarray(uvp).astype(np.float32),
        })

    _CACHED["maps"] = in_maps
    res = run_bass_kernel_spmd(nc, in_maps, list(range(8)))
    out = np.zeros((CUR, BS, DIM), np.float32)
    for c in range(8):
        bg, hg = c // 4, c % 4
        r = res.results[c]["outp"].reshape(CUR, NB, DIM)
        out[:, 2 * bg, :] += r[:, 0, :]
        out[:, 2 * bg + 1, :] += r[:, 1, :]
    return out


# revision 20
# speedup vs baseline: 1.0868x; 1.0022x over previous
"""TransformerXL relative attention on 8 TRN2 NeuronCores — v3.

Sharding: TP over heads 4-way x DP over batch 2-way.  Core c handles
batch group bg=c//4 (batches 2bg, 2bg+1) and head group hg=c%4 (4 heads,
256 head-dims).  Each core computes a partial output projection
[CUR, 2, DIM]; the host sums the 4 head-group partials per batch.

v3 vs v2 (129.3us):
- TP4xDP2 instead of TP2xDP4: the batch-independent R^T = pos @ W_pos
  projection halves (2 t-tiles instead of 4), saving 16384 PE cycles;
  everything else is work-neutral (8 (head,batch) attention units per
  core either way).
- The +u / +v query biases are folded into the Q eviction as DVE
  tensor_scalar adds (per-partition scalar AP), removing the PE
  ones-row matmuls (-4096 cycles).
- The shifted position-score readback is one 3-D AP accum DMA per unit
  ([[RSTR,128],[128*RSTR,4],[1,1024]]) instead of 4 per-qt reads,
  cutting SWDGE descriptor-gen on GPSIMD from 33us to 9us.
"""

import numpy as np
import ml_dtypes

import concourse.bass as bass
import concourse.mybir as mybir
import concourse.tile as tile
from concourse import bacc
from concourse.bass_utils import run_bass_kernel_spmd
from concourse.masks import make_identity

CUR, FULL, BS, DIM, H, D = 512, 1024, 4, 1024, 16, 64
NHC = 4                 # heads per core
NB = 2                  # batches per core
HDC = NHC * D           # 256 head-dims per core
SCALE = 1.0 / D ** 0.5  # 0.125
BIG = -30000.0
PADW = 1536             # padded row width for the shift round trip
RSTR = PADW - 1         # shifted read row stride
BF = mybir.dt.bfloat16
F32 = mybir.dt.float32
Exp = mybir.ActivationFunctionType.Exp
Copy = mybir.ActivationFunctionType.Copy
AluAdd = mybir.AluOpType.add

_CACHED = {}


def build_program():
    nc = bacc.Bacc(None, target_bir_lowering=False, debug=False)
    # One packed input tensor, phase-ordered so a handful of big DMAs
    # stream it in the order the projection loops consume it:
    #   R-block  8 k8-slices of (xpos 1024 | wpos 256)       = 8*1280
    #   Q-block  8 k8-slices of (xcur 2b*512 | wq 256)        = 8*1280
    #   K-block  8 k8-slices of (xfull 2b*1024 | wk 256)      = 8*2304
    #   V-block  8 k8-slices of (wv 256)                      = 8*256
    #   wproj    hd-major [128, 2, 1024]                      = 2048
    PCOLS = 8 * 1280 + 8 * 1280 + 8 * 2304 + 8 * 256 + 2048
    packed = nc.declare_dram_parameter("packed", [128, PCOLS], BF,
                                       isOutput=False)
    uvp_in = nc.declare_dram_parameter("uvp", [128, 4], F32, isOutput=False)
    outp = nc.declare_dram_parameter("outp", [CUR, NB * DIM], F32, isOutput=True)
    R0, Q0, K0, V0, W0 = (0, 8 * 1280, 8 * 2560, 8 * 2560 + 8 * 2304,
                          8 * 2560 + 8 * 2304 + 8 * 256)

    with tile.TileContext(nc) as tc:
        const = tc.alloc_tile_pool(name="const", bufs=1)
        psA = tc.alloc_tile_pool(name="psA", bufs=3, space="PSUM")
        psB = tc.alloc_tile_pool(name="psB", bufs=2, space="PSUM")
        psT = tc.alloc_tile_pool(name="psT", bufs=2, space="PSUM")
        psO = tc.alloc_tile_pool(name="psO", bufs=1, space="PSUM")
        slabp = tc.alloc_tile_pool(name="slabp", bufs=3)
        sp = tc.alloc_tile_pool(name="sp", bufs=3)
        atp = tc.alloc_tile_pool(name="atp", bufs=3)
        work = tc.alloc_tile_pool(name="work", bufs=2)
        dram = tc.alloc_tile_pool(name="dram", bufs=4, space="DRAM")

        # ---- resident SBUF tensors ----
        # pk_sb mirrors the packed DRAM layout 1:1
        pk_sb = const.tile([128, PCOLS], BF)
        uvp_sb = const.tile([128, 4], F32)
        ident = const.tile([128, 128], BF)
        big_sb = const.tile([128, 512], BF)
        kt_sb = const.tile([128, NB, 2, FULL], BF)   # K^T (dc, b, t, j)
        rt_sb = const.tile([128, 2, FULL], BF)       # R^T (dc, t, m)
        v_sb = const.tile([128, 8, NB, NHC, D + 1], BF)
        qt_sb = const.tile([128, NB, 2, 2, CUR], BF)  # Q^T +u/+v (dc,b,t,uv,i)
        o_sb = const.tile([128, NB, 2, CUR], BF)      # O^T normalized

        # packed views (k8-sliced)
        def xpos_v(k8):
            return pk_sb[:, R0 + k8 * 1280: R0 + k8 * 1280 + 1024]

        def wpos_v(k8, t):
            c = R0 + k8 * 1280 + 1024 + t * 128
            return pk_sb[:, c:c + 128]

        def xcur_v(k8, b):
            c = Q0 + k8 * 1280 + b * 512
            return pk_sb[:, c:c + 512]

        def wq_v(k8, t):
            c = Q0 + k8 * 1280 + 1024 + t * 128
            return pk_sb[:, c:c + 128]

        def xfull_v(k8, b, c0, c1):
            c = K0 + k8 * 2304 + b * 1024
            return pk_sb[:, c + c0:c + c1]

        def wk_v(k8, t):
            c = K0 + k8 * 2304 + 2048 + t * 128
            return pk_sb[:, c:c + 128]

        def wv_v(k8):
            c = V0 + k8 * 256
            return pk_sb[:, c:c + 256]

        def wproj_v(t, c0, c1):
            c = W0 + t * 1024
            return pk_sb[:, c + c0:c + c1]

        # phase-ordered streaming loads: few, large DMAs
        nc.sync.dma_start(out=uvp_sb[:], in_=uvp_in[:])
        bounds = [R0, R0 + 1280, R0 + 2 * 1280, R0 + 4 * 1280, R0 + 6 * 1280,
                  Q0, Q0 + 4 * 1280,
                  K0, K0 + 2304, K0 + 2 * 2304, K0 + 3 * 2304, K0 + 4 * 2304,
                  K0 + 5 * 2304, K0 + 6 * 2304, K0 + 7 * 2304,
                  V0, PCOLS]
        for c0, c1 in zip(bounds[:-1], bounds[1:]):
            nc.sync.dma_start(out=pk_sb[:, c0:c1], in_=packed[:, c0:c1])
        make_identity(nc, ident[:])
        nc.vector.memset(big_sb[:], BIG)
        nc.gpsimd.memset(v_sb[:, :, :, :, D:D + 1], 1.0)

        # shift buffers: one per unit slot (double-buffered across units);
        # pad columns [1024,1536) hold BIG once (causal mask for free).
        pdram = []
        for rep in range(4):
            t = dram.tile([CUR * PADW], BF, tag=f"pd_{rep}", name=f"pd_{rep}")
            pdram.append(t)
            for rq in range(4):
                nc.sync.dma_start(
                    out=bass.AP(tensor=t.tensor,
                                offset=rq * 128 * PADW + FULL,
                                ap=[[PADW, 128], [1, 512]]),
                    in_=big_sb[:])

        # ---- projections ----
        def stage_r_start():
            """All 4 R^T groups k8-major across 4 PSUM tiles, chasing the
            per-k8 input DMA slices."""
            tiles = [psA.tile([128, 512], F32, tag="pj", name=f"p_r0_{g}")
                     for g in range(2)]
            tiles += [psB.tile([128, 512], F32, tag="pv", name=f"p_r1_{g}")
                      for g in range(2)]
            for k8 in range(8):
                for g, pk in enumerate(tiles):
                    t, nh = g // 2, g % 2
                    nc.tensor.matmul(
                        pk[:], wpos_v(k8, t),
                        xpos_v(k8)[:, nh * 512:(nh + 1) * 512],
                        start=(k8 == 0), stop=(k8 == 7))
            for g, pk in enumerate(tiles):
                t, nh = g // 2, g % 2
                nc.vector.tensor_copy(
                    rt_sb[:, t, nh * 512:(nh + 1) * 512], pk[:])

        def stage_q():
            """Q k8-major across 4 PSUM tiles, chasing the xcur slices."""
            tiles = [psA.tile([128, 512], F32, tag="pj", name=f"p_q_{g}")
                     for g in range(2)]
            tiles += [psB.tile([128, 512], F32, tag="pv", name=f"p_q1_{g}")
                      for g in range(2)]
            for k8 in range(8):
                for g, pq in enumerate(tiles):
                    b, t = g // 2, g % 2
                    nc.tensor.matmul(
                        pq[:], wq_v(k8, t), xcur_v(k8, b),
                        start=(k8 == 0), stop=(k8 == 7))
            for g, pq in enumerate(tiles):
                b, t = g // 2, g % 2
                nc.vector.tensor_scalar_add(
                    qt_sb[:, b, t, 0, :], pq[:], uvp_sb[:, 2 * t:2 * t + 1])
                nc.vector.tensor_scalar_add(
                    qt_sb[:, b, t, 1, :], pq[:],
                    uvp_sb[:, 2 * t + 1:2 * t + 2])

        def pos_unit(u):
            for qt in range(4):
                a_pos_qt(u, qt)

        def stage_k(b):
            """K groups sequential (xfull slices stream in during R/Q)."""
            if True:
                for t in range(2):
                    for nh in range(2):
                        pk = psA.tile([128, 512], F32, tag="pj",
                                      name=f"p_k_{b}{t}{nh}")
                        for k8 in range(8):
                            nc.tensor.matmul(
                                pk[:], wk_v(k8, t),
                                xfull_v(k8, b, nh * 512, (nh + 1) * 512),
                                start=(k8 == 0), stop=(k8 == 7))
                        nc.vector.tensor_copy(
                            kt_sb[:, b, t, nh * 512:(nh + 1) * 512], pk[:])

        def v_group(b, jt):
            pv = psB.tile([128, HDC], F32, tag="pv", name=f"p_v_{b}{jt}")
            for k8 in range(8):
                nc.tensor.matmul(
                    pv[:], xfull_v(k8, b, jt * 128, (jt + 1) * 128),
                    wv_v(k8),
                    start=(k8 == 0), stop=(k8 == 7))
            nc.vector.tensor_copy(
                v_sb[:, jt, b, :, 0:D],
                pv[:].rearrange("p (h d) -> p h d", h=NHC))

        # ---- attention stages (per unit u = (h, b)) ----
        s_tiles = {}    # u -> content+shifted-pos scores [128, 4, FULL]
        at_tiles = {}   # u -> [128, 8, CUR] A^T blocks
        slab_tiles = {}
        ov_tiles = {}

        def hb(u):
            return u % 4, u // 4

        def evict(engine, out, in_):
            if engine == "v":
                nc.vector.tensor_copy(out, in_)
            else:
                nc.scalar.activation(out, in_, Copy)

        SLAB_ENG = {qt: "v" for qt in range(4)}
        CONT_ENG = {(0, 0): "a", (1, 0): "a", (2, 0): "a", (3, 0): "a",
                    (0, 1): "a", (1, 1): "a", (2, 1): "v", (3, 1): "v"}

        def a_pos_qt(u, qt):
            """Position scores for one query tile: matmul, evict, write."""
            h, b = hb(u)
            p0 = (h % 2) * 64
            th = h // 2
            if qt == 0:
                slab_tiles[u] = slabp.tile([128, 4, FULL], BF, tag="slab",
                                           name=f"slab_{u}")
            slab = slab_tiles[u]
            m_min = 384 - 128 * qt
            c = m_min
            ci = 0
            while c < 1024:
                ce = min(c + 512, 1024)
                pp = psA.tile([128, ce - c], F32, tag="pj",
                              name=f"pp_{u}_{qt}_{ci}")
                nc.tensor.matmul(
                    pp[:],
                    qt_sb[p0:p0 + 64, b, th, 1, qt * 128:(qt + 1) * 128],
                    rt_sb[p0:p0 + 64, th, c:ce],
                    start=True, stop=True)
                evict(SLAB_ENG[qt], slab[:, qt, c:ce], pp[:])
                c = ce
                ci += 1
            pd = pdram[u % 4]
            nc.sync.dma_start(
                out=bass.AP(tensor=pd.tensor,
                            offset=qt * 128 * PADW + m_min,
                            ap=[[PADW, 128], [1, 1024 - m_min]]),
                in_=slab[:, qt, m_min:1024])

        def a_content_qt(u, qt):
            """Content scores for one tile + shifted-pos accumulate DMA."""
            h, b = hb(u)
            p0 = (h % 2) * 64
            th = h // 2
            pd = pdram[u % 4]
            if qt == 0:
                s_tiles[u] = sp.tile([128, 4, FULL], BF, tag="s", name=f"s_{u}")
            s_all = s_tiles[u]
            jw = 640 + 128 * qt
            c = 0
            ci = 0
            while c < jw:
                ce = min(c + 512, jw)
                pc = psB.tile([128, ce - c], F32, tag="pv",
                              name=f"pc_{u}_{qt}_{ci}")
                nc.tensor.matmul(
                    pc[:],
                    qt_sb[p0:p0 + 64, b, th, 0, qt * 128:(qt + 1) * 128],
                    kt_sb[p0:p0 + 64, b, th, c:ce],
                    start=True, stop=True)
                evict(CONT_ENG[(qt, ci)], s_all[:, qt, c:ce], pc[:])
                c = ce
                ci += 1
            # shifted position rows accumulate onto the content scores
            nc.gpsimd.dma_start(
                out=s_all[:, qt, 0:jw],
                in_=bass.AP(tensor=pd.tensor,
                            offset=qt * 128 * PADW + 511 - 128 * qt,
                            ap=[[RSTR, 128], [1, jw]]),
                accum_op=AluAdd)

        def e_qt(u, qt, tpool=None):
            """Transpose hull blocks of one tile, exponentiate into A^T."""
            if qt == 0:
                at_tiles[u] = atp.tile([128, 8, CUR], BF, tag="at",
                                       name=f"at_{u}")
            at_all = at_tiles[u]
            s_all = s_tiles[u]
            nj8 = qt + 5
            st = (tpool or psT).tile([128, 8, 128], BF,
                                     tag="pj" if tpool else "pt",
                                     name=f"st_{u}_{qt}")
            for j8 in range(nj8):
                nc.tensor.transpose(st[:, j8, :],
                                    s_all[:, qt, j8 * 128:(j8 + 1) * 128],
                                    ident[:])
            nc.scalar.activation(
                at_all[:, 0:nj8, qt * 128:(qt + 1) * 128],
                st[:, 0:nj8, :], Exp, scale=SCALE)

        def f_av(u, jts, pool=None):
            """Part of A^T @ V accumulation (ones column -> denominator)."""
            h, b = hb(u)
            if jts[0] == 0:
                ov_tiles[u] = (pool or psO).tile(
                    [D + 1, CUR], F32, tag="pv" if pool else "po",
                    name=f"ov_{u}")
            ov = ov_tiles[u]
            at_all = at_tiles[u]
            for jt in jts:
                c0 = max(0, (jt - 4) * 128)
                nc.tensor.matmul(ov[:, c0:], v_sb[:, jt, b, h, :],
                                 at_all[:, jt, c0:],
                                 start=(jt == 0), stop=(jt == 7),
                                 skip_group_check=True)

        def f_norm(u, chunks=((0, CUR),), done=True):
            """Normalize by the softmax denominator into O^T."""
            h, b = hb(u)
            p0 = (h % 2) * 64
            th = h // 2
            ov = ov_tiles[u]
            if done:
                ov_tiles.pop(u)
                at_tiles.pop(u, None)
            for c0, c1 in chunks:
                rden = work.tile([1, CUR], F32, tag="rden", bufs=2,
                                 name=f"rden_{u}_{c0}")
                nc.vector.reciprocal(rden[0:1, 0:c1 - c0], ov[D:D + 1, c0:c1])
                rdb = work.tile([64, CUR], F32, tag="rdb", bufs=2,
                                name=f"rdb_{u}_{c0}")
                nc.gpsimd.partition_broadcast(rdb[0:64, 0:c1 - c0],
                                              rden[0:1, 0:c1 - c0])
                nc.vector.tensor_mul(o_sb[p0:p0 + 64, b, th, c0:c1],
                                     ov[0:D, c0:c1], rdb[0:64, 0:c1 - c0])

        def unit_slot(up, uc, ue, vgs=(), projqts=()):
            """One pipeline slot: position scores for unit up, content
            scores for unit uc, transpose/exp/AV for unit ue, plus
            V-projection or output-projection fillers."""
            vit = iter(vgs)
            pit = iter(projqts)
            for qt in range(4):
                if ue is not None:
                    e_qt(ue, qt)
                v = next(vit, None)
                if v is not None:
                    v_group(*v)
                if up is not None:
                    a_pos_qt(up, qt)
                if qt % 2 == 1:
                    p = next(pit, None)
                    if p is not None:
                        proj_qt(*p, pools=((psA, "pj"), (psA, "pj")))
            for qt in range(4):
                v = next(vit, None)
                if v is not None:
                    v_group(*v)
                if uc is not None:
                    a_content_qt(uc, qt)
                if ue is not None and qt % 2 == 1:
                    f_av(ue, ((qt - 1) * 2, (qt - 1) * 2 + 1,
                              (qt - 1) * 2 + 2, (qt - 1) * 2 + 3))
            if ue is not None:
                f_norm(ue)

        # ---- output projection ----
        proj_tiles = {}

        def proj_start(b, qt, pools=None, nhs=(0, 1)):
            for nh in nhs:
                pool, tg = (pools or ((psB, "pv"), (psO, "po")))[nh]
                pr = pool.tile([128, 512], F32, tag=tg, name=f"pr_{b}{qt}{nh}")
                proj_tiles[(b, qt, nh)] = pr
                nc.tensor.matmul(
                    pr[:], o_sb[:, b, 0, qt * 128:(qt + 1) * 128],
                    wproj_v(0, nh * 512, (nh + 1) * 512),
                    start=True, stop=False)

        def proj_finish(b, qt):
            ot = work.tile([128, FULL], F32, tag="ot", bufs=2,
                           name=f"ot_{b}{qt}")
            for nh in range(2):
                if (b, qt, nh) not in proj_tiles:
                    proj_start(b, qt, nhs=(nh,))
                pr = proj_tiles.pop((b, qt, nh))
                nc.tensor.matmul(
                    pr[:], o_sb[:, b, 1, qt * 128:(qt + 1) * 128],
                    wproj_v(1, nh * 512, (nh + 1) * 512),
                    start=False, stop=True)
                evict("a" if nh == 0 else "v",
                      ot[:, nh * 512:(nh + 1) * 512], pr[:])
                nc.sync.dma_start(
                    out=outp[qt * 128:(qt + 1) * 128,
                             b * DIM + nh * 512:b * DIM + (nh + 1) * 512],
                    in_=ot[:, nh * 512:(nh + 1) * 512])

        def proj_qt(b, qt, pools=None):
            proj_start(b, qt, pools=pools)
            proj_finish(b, qt)

        # ---- schedule ----
        # 3-phase software pipeline (pos / content / ef decoupled).
        # pos(0..3) run right after R+Q, inside the DMA-bound startup
        # window, before the xfull-dependent K/V matmuls enter the PE
        # queue; content follows per-batch K; ef trails by 2 slots.
        vlist = [(b, jt) for b in range(NB) for jt in range(8)]
        stage_r_start()
        stage_q()
        stage_k(0)
        stage_k(1)
        unit_slot(0, 0, None, vgs=vlist[0:4])
        unit_slot(1, 1, None, vgs=vlist[4:12])
        unit_slot(2, 2, 0, vgs=vlist[12:16])
        for u in range(3, 6):
            unit_slot(u, u, u - 2)
        unit_slot(6, 6, 4, projqts=((0, 0), (0, 1)))
        unit_slot(7, 7, 5, projqts=((0, 2), (0, 3)))
        # tail: interleave the last two units (both batch 1)
        u6, u7 = 6, 7
        for qt in range(4):
            e_qt(u6, qt)
        e_qt(u7, 0)
        e_qt(u7, 1)
        f_av(u6, (0, 1, 2, 3))
        e_qt(u7, 2)
        f_av(u6, (4, 5, 6, 7))
        e_qt(u7, 3)
        proj_start(1, 0, pools=((psA, "pj"), (psA, "pj")))
        f_norm(u6)
        f_av(u7, (0, 1, 2, 3), pool=psB)
        f_av(u7, (4, 5, 6, 7), pool=psB)

        for qt in range(4):
            f_norm(u7, chunks=((qt * 128, (qt + 1) * 128),), done=(qt == 3))
            if qt == 0:
                proj_finish(1, 0)
            else:
                proj_qt(1, qt)

        for p in (dram, work, atp, sp, slabp, psO, psT, psB, psA, const):
            p.release()
    nc.compile()
    return nc


def kernel(inputs, pos_embedding, full_input, u, v, W_kv, b_kv, W_q, b_q,
           W_pos, b_pos, W_proj, b_proj, mask):
    bf = ml_dtypes.bfloat16
    inputs = np.asarray(inputs)
    full_input = np.asarray(full_input)
    pos = np.asarray(pos_embedding)[:, 0, :]

    if "nc" not in _CACHED:
        _CACHED["nc"] = build_program()
    nc = _CACHED["nc"]

    PCOLS = 8 * 1280 + 8 * 1280 + 8 * 2304 + 8 * 256 + 2048
    R0, Q0, K0, V0, W0 = (0, 8 * 1280, 8 * 2560, 8 * 2560 + 8 * 2304,
                          8 * 2560 + 8 * 2304 + 8 * 256)
    posT = pos.T.astype(bf)                      # [DIM, FULL]
    in_maps = []
    for c in range(8):
        bg, hg = c // 4, c % 4
        sl = slice(hg * HDC, (hg + 1) * HDC)
        bsl = slice(2 * bg, 2 * bg + 2)
        uvec = (np.asarray(u).reshape(-1) + np.asarray(b_q))[sl]
        vvec = (np.asarray(v).reshape(-1) + np.asarray(b_q))[sl]
        # uvp[p, 2t+0/1] = u/v for head-dim t*128+p
        uvp = np.stack([uvec[0:128], vvec[0:128],
                        uvec[128:256], vvec[128:256]], axis=1)
        xf = full_input[:, bsl, :].transpose(2, 1, 0).astype(bf)  # [DIM,2,FULL]
        xc = inputs[:, bsl, :].transpose(2, 1, 0).astype(bf)      # [DIM,2,CUR]
        wq = W_q[:, sl].astype(bf)
        wk = W_kv[:, hg * HDC:(hg + 1) * HDC].astype(bf)
        wv = W_kv[:, H * D + hg * HDC:H * D + (hg + 1) * HDC].astype(bf)
        wpos = W_pos[:, sl].astype(bf)
        wproj = W_proj[sl, :].astype(bf)          # [HDC, DIM]

        P = np.zeros((128, PCOLS), bf)
        for k8 in range(8):
            dsl = slice(k8 * 128, (k8 + 1) * 128)
            P[:, R0 + k8 * 1280:R0 + k8 * 1280 + 1024] = posT[dsl]
            P[:, R0 + k8 * 1280 + 1024:R0 + (k8 + 1) * 1280] = wpos[dsl]
            P[:, Q0 + k8 * 1280:Q0 + k8 * 1280 + 1024] = \
                xc[dsl].reshape(128, NB * CUR)
            P[:, Q0 + k8 * 1280 + 1024:Q0 + (k8 + 1) * 1280] = wq[dsl]
            P[:, K0 + k8 * 2304:K0 + k8 * 2304 + 2048] = \
                xf[dsl].reshape(128, NB * FULL)
            P[:, K0 + k8 * 2304 + 2048:K0 + (k8 + 1) * 2304] = wk[dsl]
            P[:, V0 + k8 * 256:V0 + (k8 + 1) * 256] = wv[dsl]
        P[:, W0:W0 + 1024] = wproj[0:128]
        P[:, W0 + 1024:W0 + 2048] = wproj[128:256]
        in_maps.append({
            "packed": P,
            "uvp": np.ascontiguousarray(uvp).astype(np.float32),
        })

    _CACHED["maps"] = in_maps
    res = run_bass_kernel_spmd(nc, in_maps, list(range(8)))
    out = np.zeros((CUR, BS, DIM), np.float32)
    for c in range(8):
        bg, hg = c // 4, c % 4
        r = res.results[c]["outp"].reshape(CUR, NB, DIM)
        out[:, 2 * bg, :] += r[:, 0, :]
        out[:, 2 * bg + 1, :] += r[:, 1, :]
    return out


# revision 21
# speedup vs baseline: 1.0987x; 1.0109x over previous
"""TransformerXL relative attention on 8 TRN2 NeuronCores — v3.

Sharding: TP over heads 4-way x DP over batch 2-way.  Core c handles
batch group bg=c//4 (batches 2bg, 2bg+1) and head group hg=c%4 (4 heads,
256 head-dims).  Each core computes a partial output projection
[CUR, 2, DIM]; the host sums the 4 head-group partials per batch.

v3 vs v2 (129.3us):
- TP4xDP2 instead of TP2xDP4: the batch-independent R^T = pos @ W_pos
  projection halves (2 t-tiles instead of 4), saving 16384 PE cycles;
  everything else is work-neutral (8 (head,batch) attention units per
  core either way).
- The +u / +v query biases are folded into the Q eviction as DVE
  tensor_scalar adds (per-partition scalar AP), removing the PE
  ones-row matmuls (-4096 cycles).
- The shifted position-score readback is one 3-D AP accum DMA per unit
  ([[RSTR,128],[128*RSTR,4],[1,1024]]) instead of 4 per-qt reads,
  cutting SWDGE descriptor-gen on GPSIMD from 33us to 9us.
"""

import numpy as np
import ml_dtypes

import concourse.bass as bass
import concourse.mybir as mybir
import concourse.tile as tile
from concourse import bacc
from concourse.bass_utils import run_bass_kernel_spmd
from concourse.masks import make_identity

CUR, FULL, BS, DIM, H, D = 512, 1024, 4, 1024, 16, 64
NHC = 4                 # heads per core
NB = 2                  # batches per core
HDC = NHC * D           # 256 head-dims per core
SCALE = 1.0 / D ** 0.5  # 0.125
BIG = -30000.0
PADW = 1536             # padded row width for the shift round trip
RSTR = PADW - 1         # shifted read row stride
BF = mybir.dt.bfloat16
F32 = mybir.dt.float32
Exp = mybir.ActivationFunctionType.Exp
Copy = mybir.ActivationFunctionType.Copy
AluAdd = mybir.AluOpType.add

_CACHED = {}


def build_program():
    nc = bacc.Bacc(None, target_bir_lowering=False, debug=False)
    # One packed input tensor, phase-ordered so a handful of big DMAs
    # stream it in the order the projection loops consume it:
    #   R-block  8 k8-slices of (xpos 1024 | wpos 256)       = 8*1280
    #   Q-block  8 k8-slices of (xcur 2b*512 | wq 256)        = 8*1280
    #   K-block  8 k8-slices of (xfull 2b*1024 | wk 256)      = 8*2304
    #   V-block  8 k8-slices of (wv 256)                      = 8*256
    #   wproj    hd-major [128, 2, 1024]                      = 2048
    PCOLS = 8 * 1280 + 8 * 1280 + 8 * 2304 + 8 * 256 + 2048
    packed = nc.declare_dram_parameter("packed", [128, PCOLS], BF,
                                       isOutput=False)
    uvp_in = nc.declare_dram_parameter("uvp", [128, 4], F32, isOutput=False)
    outp = nc.declare_dram_parameter("outp", [CUR, NB * DIM], F32, isOutput=True)
    R0, Q0, K0, V0, W0 = (0, 8 * 1280, 8 * 2560, 8 * 2560 + 8 * 2304,
                          8 * 2560 + 8 * 2304 + 8 * 256)

    with tile.TileContext(nc) as tc:
        const = tc.alloc_tile_pool(name="const", bufs=1)
        psA = tc.alloc_tile_pool(name="psA", bufs=3, space="PSUM")
        psB = tc.alloc_tile_pool(name="psB", bufs=2, space="PSUM")
        psT = tc.alloc_tile_pool(name="psT", bufs=2, space="PSUM")
        psO = tc.alloc_tile_pool(name="psO", bufs=1, space="PSUM")
        slabp = tc.alloc_tile_pool(name="slabp", bufs=2)
        sp = tc.alloc_tile_pool(name="sp", bufs=2)
        atp = tc.alloc_tile_pool(name="atp", bufs=3)
        work = tc.alloc_tile_pool(name="work", bufs=2)
        dram = tc.alloc_tile_pool(name="dram", bufs=2, space="DRAM")

        # ---- resident SBUF tensors ----
        # pk_sb mirrors the packed DRAM layout 1:1
        pk_sb = const.tile([128, PCOLS], BF)
        uvp_sb = const.tile([128, 4], F32)
        ident = const.tile([128, 128], BF)
        big_sb = const.tile([128, 512], BF)
        kt_sb = const.tile([128, NB, 2, FULL], BF)   # K^T (dc, b, t, j)
        rt_sb = const.tile([128, 2, FULL], BF)       # R^T (dc, t, m)
        v_sb = const.tile([128, 8, NB, NHC, D + 1], BF)
        qt_sb = const.tile([128, NB, 2, 2, CUR], BF)  # Q^T +u/+v (dc,b,t,uv,i)
        o_sb = const.tile([128, NB, 2, CUR], BF)      # O^T normalized

        # packed views (k8-sliced)
        def xpos_v(k8):
            return pk_sb[:, R0 + k8 * 1280: R0 + k8 * 1280 + 1024]

        def wpos_v(k8, t):
            c = R0 + k8 * 1280 + 1024 + t * 128
            return pk_sb[:, c:c + 128]

        def xcur_v(k8, b):
            c = Q0 + k8 * 1280 + b * 512
            return pk_sb[:, c:c + 512]

        def wq_v(k8, t):
            c = Q0 + k8 * 1280 + 1024 + t * 128
            return pk_sb[:, c:c + 128]

        def xfull_v(k8, b, c0, c1):
            c = K0 + k8 * 2304 + b * 1024
            return pk_sb[:, c + c0:c + c1]

        def wk_v(k8, t):
            c = K0 + k8 * 2304 + 2048 + t * 128
            return pk_sb[:, c:c + 128]

        def wv_v(k8):
            c = V0 + k8 * 256
            return pk_sb[:, c:c + 256]

        def wproj_v(t, c0, c1):
            c = W0 + t * 1024
            return pk_sb[:, c + c0:c + c1]

        # phase-ordered streaming loads: few, large DMAs
        nc.sync.dma_start(out=uvp_sb[:], in_=uvp_in[:])
        bounds = [R0, R0 + 1280, R0 + 2 * 1280, R0 + 4 * 1280, R0 + 6 * 1280,
                  Q0, Q0 + 4 * 1280,
                  K0, K0 + 2304, K0 + 2 * 2304, K0 + 3 * 2304, K0 + 4 * 2304,
                  K0 + 5 * 2304, K0 + 6 * 2304, K0 + 7 * 2304,
                  V0, PCOLS]
        for c0, c1 in zip(bounds[:-1], bounds[1:]):
            nc.sync.dma_start(out=pk_sb[:, c0:c1], in_=packed[:, c0:c1])
        make_identity(nc, ident[:])
        nc.vector.memset(big_sb[:], BIG)
        nc.gpsimd.memset(v_sb[:, :, :, :, D:D + 1], 1.0)

        # shift buffers: one per unit slot (double-buffered across units);
        # pad columns [1024,1536) hold BIG once (causal mask for free).
        pdram = []
        for rep in range(2):
            t = dram.tile([CUR * PADW], BF, tag=f"pd_{rep}", name=f"pd_{rep}")
            pdram.append(t)
            for rq in range(4):
                nc.sync.dma_start(
                    out=bass.AP(tensor=t.tensor,
                                offset=rq * 128 * PADW + FULL,
                                ap=[[PADW, 128], [1, 512]]),
                    in_=big_sb[:])

        # ---- projections ----
        def stage_r_start():
            """All 4 R^T groups k8-major across 4 PSUM tiles, chasing the
            per-k8 input DMA slices."""
            tiles = [psA.tile([128, 512], F32, tag="pj", name=f"p_r0_{g}")
                     for g in range(2)]
            tiles += [psB.tile([128, 512], F32, tag="pv", name=f"p_r1_{g}")
                      for g in range(2)]
            for k8 in range(8):
                for g, pk in enumerate(tiles):
                    t, nh = g // 2, g % 2
                    nc.tensor.matmul(
                        pk[:], wpos_v(k8, t),
                        xpos_v(k8)[:, nh * 512:(nh + 1) * 512],
                        start=(k8 == 0), stop=(k8 == 7))
            for g, pk in enumerate(tiles):
                t, nh = g // 2, g % 2
                nc.vector.tensor_copy(
                    rt_sb[:, t, nh * 512:(nh + 1) * 512], pk[:])

        def stage_q():
            """Q k8-major across 4 PSUM tiles, chasing the xcur slices."""
            tiles = [psA.tile([128, 512], F32, tag="pj", name=f"p_q_{g}")
                     for g in range(2)]
            tiles += [psB.tile([128, 512], F32, tag="pv", name=f"p_q1_{g}")
                      for g in range(2)]
            for k8 in range(8):
                for g, pq in enumerate(tiles):
                    b, t = g // 2, g % 2
                    nc.tensor.matmul(
                        pq[:], wq_v(k8, t), xcur_v(k8, b),
                        start=(k8 == 0), stop=(k8 == 7))
            for g, pq in enumerate(tiles):
                b, t = g // 2, g % 2
                nc.vector.tensor_scalar_add(
                    qt_sb[:, b, t, 0, :], pq[:], uvp_sb[:, 2 * t:2 * t + 1])
                nc.vector.tensor_scalar_add(
                    qt_sb[:, b, t, 1, :], pq[:],
                    uvp_sb[:, 2 * t + 1:2 * t + 2])

        def pos_unit(u):
            for qt in range(4):
                a_pos_qt(u, qt)

        def stage_k(b):
            """K groups sequential (xfull slices stream in during R/Q)."""
            if True:
                for t in range(2):
                    for nh in range(2):
                        pk = psA.tile([128, 512], F32, tag="pj",
                                      name=f"p_k_{b}{t}{nh}")
                        for k8 in range(8):
                            nc.tensor.matmul(
                                pk[:], wk_v(k8, t),
                                xfull_v(k8, b, nh * 512, (nh + 1) * 512),
                                start=(k8 == 0), stop=(k8 == 7))
                        nc.vector.tensor_copy(
                            kt_sb[:, b, t, nh * 512:(nh + 1) * 512], pk[:])

        def v_group(b, jt):
            pv = psB.tile([128, HDC], F32, tag="pv", name=f"p_v_{b}{jt}")
            for k8 in range(8):
                nc.tensor.matmul(
                    pv[:], xfull_v(k8, b, jt * 128, (jt + 1) * 128),
                    wv_v(k8),
                    start=(k8 == 0), stop=(k8 == 7))
            nc.vector.tensor_copy(
                v_sb[:, jt, b, :, 0:D],
                pv[:].rearrange("p (h d) -> p h d", h=NHC))

        # ---- attention stages (per unit u = (h, b)) ----
        s_tiles = {}    # u -> content+shifted-pos scores [128, 4, FULL]
        at_tiles = {}   # u -> [128, 8, CUR] A^T blocks
        slab_tiles = {}
        ov_tiles = {}

        def hb(u):
            return u % 4, u // 4

        def evict(engine, out, in_):
            if engine == "v":
                nc.vector.tensor_copy(out, in_)
            else:
                nc.scalar.activation(out, in_, Copy)

        SLAB_ENG = {qt: "v" for qt in range(4)}
        CONT_ENG = {(0, 0): "a", (1, 0): "a", (2, 0): "a", (3, 0): "a",
                    (0, 1): "a", (1, 1): "a", (2, 1): "v", (3, 1): "v"}

        def a_pos_qt(u, qt):
            """Position scores for one query tile: matmul, evict, write."""
            h, b = hb(u)
            p0 = (h % 2) * 64
            th = h // 2
            if qt == 0:
                slab_tiles[u] = slabp.tile([128, 4, FULL], BF, tag="slab",
                                           name=f"slab_{u}")
            slab = slab_tiles[u]
            m_min = 384 - 128 * qt
            c = m_min
            ci = 0
            while c < 1024:
                ce = min(c + 512, 1024)
                pp = psA.tile([128, ce - c], F32, tag="pj",
                              name=f"pp_{u}_{qt}_{ci}")
                nc.tensor.matmul(
                    pp[:],
                    qt_sb[p0:p0 + 64, b, th, 1, qt * 128:(qt + 1) * 128],
                    rt_sb[p0:p0 + 64, th, c:ce],
                    start=True, stop=True)
                evict(SLAB_ENG[qt], slab[:, qt, c:ce], pp[:])
                c = ce
                ci += 1
            pd = pdram[u % 2]
            nc.sync.dma_start(
                out=bass.AP(tensor=pd.tensor,
                            offset=qt * 128 * PADW + m_min,
                            ap=[[PADW, 128], [1, 1024 - m_min]]),
                in_=slab[:, qt, m_min:1024])

        def a_content_qt(u, qt):
            """Content scores for one tile + shifted-pos accumulate DMA."""
            h, b = hb(u)
            p0 = (h % 2) * 64
            th = h // 2
            pd = pdram[u % 2]
            if qt == 0:
                s_tiles[u] = sp.tile([128, 4, FULL], BF, tag="s", name=f"s_{u}")
            s_all = s_tiles[u]
            jw = 640 + 128 * qt
            c = 0
            ci = 0
            while c < jw:
                ce = min(c + 512, jw)
                pc = psB.tile([128, ce - c], F32, tag="pv",
                              name=f"pc_{u}_{qt}_{ci}")
                nc.tensor.matmul(
                    pc[:],
                    qt_sb[p0:p0 + 64, b, th, 0, qt * 128:(qt + 1) * 128],
                    kt_sb[p0:p0 + 64, b, th, c:ce],
                    start=True, stop=True)
                evict(CONT_ENG[(qt, ci)], s_all[:, qt, c:ce], pc[:])
                c = ce
                ci += 1
            # shifted position rows accumulate onto the content scores
            nc.gpsimd.dma_start(
                out=s_all[:, qt, 0:jw],
                in_=bass.AP(tensor=pd.tensor,
                            offset=qt * 128 * PADW + 511 - 128 * qt,
                            ap=[[RSTR, 128], [1, jw]]),
                accum_op=AluAdd)

        def e_qt(u, qt, tpool=None):
            """Transpose hull blocks of one tile, exponentiate into A^T."""
            if qt == 0:
                at_tiles[u] = atp.tile([128, 8, CUR], BF, tag="at",
                                       name=f"at_{u}")
            at_all = at_tiles[u]
            s_all = s_tiles[u]
            nj8 = qt + 5
            st = (tpool or psT).tile([128, 8, 128], BF,
                                     tag="pj" if tpool else "pt",
                                     name=f"st_{u}_{qt}")
            for j8 in range(nj8):
                nc.tensor.transpose(st[:, j8, :],
                                    s_all[:, qt, j8 * 128:(j8 + 1) * 128],
                                    ident[:])
            nc.scalar.activation(
                at_all[:, 0:nj8, qt * 128:(qt + 1) * 128],
                st[:, 0:nj8, :], Exp, scale=SCALE)

        def f_av(u, jts, pool=None):
            """Part of A^T @ V accumulation (ones column -> denominator)."""
            h, b = hb(u)
            if jts[0] == 0:
                ov_tiles[u] = (pool or psO).tile(
                    [D + 1, CUR], F32, tag="pv" if pool else "po",
                    name=f"ov_{u}")
            ov = ov_tiles[u]
            at_all = at_tiles[u]
            for jt in jts:
                c0 = max(0, (jt - 4) * 128)
                nc.tensor.matmul(ov[:, c0:], v_sb[:, jt, b, h, :],
                                 at_all[:, jt, c0:],
                                 start=(jt == 0), stop=(jt == 7),
                                 skip_group_check=True)

        def f_norm(u, chunks=((0, CUR),), done=True):
            """Normalize by the softmax denominator into O^T."""
            h, b = hb(u)
            p0 = (h % 2) * 64
            th = h // 2
            ov = ov_tiles[u]
            if done:
                ov_tiles.pop(u)
                at_tiles.pop(u, None)
            for c0, c1 in chunks:
                rden = work.tile([1, CUR], F32, tag="rden", bufs=3,
                                 name=f"rden_{u}_{c0}")
                nc.vector.reciprocal(rden[0:1, 0:c1 - c0], ov[D:D + 1, c0:c1])
                rdb = work.tile([64, CUR], F32, tag="rdb", bufs=3,
                                name=f"rdb_{u}_{c0}")
                nc.gpsimd.partition_broadcast(rdb[0:64, 0:c1 - c0],
                                              rden[0:1, 0:c1 - c0])
                nc.vector.tensor_mul(o_sb[p0:p0 + 64, b, th, c0:c1],
                                     ov[0:D, c0:c1], rdb[0:64, 0:c1 - c0])

        def unit_slot(up, uc, ue, vgs=(), projqts=()):
            """One pipeline slot: position scores for unit up, content
            scores for unit uc, transpose/exp/AV for unit ue, plus
            V-projection or output-projection fillers."""
            vit = iter(vgs)
            pit = iter(projqts)
            for qt in range(4):
                if ue is not None:
                    e_qt(ue, qt)
                v = next(vit, None)
                if v is not None:
                    v_group(*v)
                if up is not None:
                    a_pos_qt(up, qt)
                if qt % 2 == 1:
                    p = next(pit, None)
                    if p is not None:
                        proj_qt(*p, pools=((psA, "pj"), (psA, "pj")))
            for qt in range(4):
                v = next(vit, None)
                if v is not None:
                    v_group(*v)
                if uc is not None:
                    a_content_qt(uc, qt)
                if ue is not None and qt % 2 == 1:
                    f_av(ue, ((qt - 1) * 2, (qt - 1) * 2 + 1,
                              (qt - 1) * 2 + 2, (qt - 1) * 2 + 3))
            if ue is not None:
                f_norm(ue)

        # ---- output projection ----
        proj_tiles = {}

        def proj_start(b, qt, pools=None, nhs=(0, 1)):
            for nh in nhs:
                pool, tg = (pools or ((psB, "pv"), (psO, "po")))[nh]
                pr = pool.tile([128, 512], F32, tag=tg, name=f"pr_{b}{qt}{nh}")
                proj_tiles[(b, qt, nh)] = pr
                nc.tensor.matmul(
                    pr[:], o_sb[:, b, 0, qt * 128:(qt + 1) * 128],
                    wproj_v(0, nh * 512, (nh + 1) * 512),
                    start=True, stop=False)

        def proj_finish(b, qt):
            ot = work.tile([128, FULL], F32, tag="ot", bufs=4,
                           name=f"ot_{b}{qt}")
            for nh in range(2):
                if (b, qt, nh) not in proj_tiles:
                    proj_start(b, qt, nhs=(nh,))
                pr = proj_tiles.pop((b, qt, nh))
                nc.tensor.matmul(
                    pr[:], o_sb[:, b, 1, qt * 128:(qt + 1) * 128],
                    wproj_v(1, nh * 512, (nh + 1) * 512),
                    start=False, stop=True)
                evict("a" if nh == 0 else "v",
                      ot[:, nh * 512:(nh + 1) * 512], pr[:])
                nc.sync.dma_start(
                    out=outp[qt * 128:(qt + 1) * 128,
                             b * DIM + nh * 512:b * DIM + (nh + 1) * 512],
                    in_=ot[:, nh * 512:(nh + 1) * 512])

        def proj_qt(b, qt, pools=None):
            proj_start(b, qt, pools=pools)
            proj_finish(b, qt)

        # ---- schedule ----
        # 3-phase software pipeline (pos / content / ef decoupled).
        # pos(0..3) run right after R+Q, inside the DMA-bound startup
        # window, before the xfull-dependent K/V matmuls enter the PE
        # queue; content follows per-batch K; ef trails by 2 slots.
        vlist = [(b, jt) for b in range(NB) for jt in range(8)]
        stage_r_start()
        stage_q()
        stage_k(0)
        stage_k(1)
        unit_slot(0, 0, None, vgs=vlist[0:4])
        unit_slot(1, 1, None, vgs=vlist[4:12])
        unit_slot(2, 2, 0, vgs=vlist[12:16])
        for u in range(3, 6):
            unit_slot(u, u, u - 2)
        unit_slot(6, 6, 4, projqts=((0, 0), (0, 1)))
        unit_slot(7, 7, 5, projqts=((0, 2), (0, 3)))
        # tail: interleave the last two units (both batch 1)
        u6, u7 = 6, 7
        for qt in range(4):
            e_qt(u6, qt)
        e_qt(u7, 0)
        e_qt(u7, 1)
        f_av(u6, (0, 1, 2, 3))
        e_qt(u7, 2)
        f_av(u6, (4, 5, 6, 7))
        e_qt(u7, 3)
        proj_start(1, 0, pools=((psA, "pj"), (psA, "pj")))
        f_norm(u6)
        f_av(u7, (0, 1, 2, 3), pool=psB)
        f_av(u7, (4, 5, 6, 7), pool=psB)

        for qt in range(4):
            f_norm(u7, chunks=((qt * 128, (qt + 1) * 128),), done=(qt == 3))
            if qt == 0:
                proj_finish(1, 0)
            else:
                proj_qt(1, qt)

        for p in (dram, work, atp, sp, slabp, psO, psT, psB, psA, const):
            p.release()
    nc.compile()
    return nc


def kernel(inputs, pos_embedding, full_input, u, v, W_kv, b_kv, W_q, b_q,
           W_pos, b_pos, W_proj, b_proj, mask):
    bf = ml_dtypes.bfloat16
    inputs = np.asarray(inputs)
    full_input = np.asarray(full_input)
    pos = np.asarray(pos_embedding)[:, 0, :]

    if "nc" not in _CACHED:
        _CACHED["nc"] = build_program()
    nc = _CACHED["nc"]

    PCOLS = 8 * 1280 + 8 * 1280 + 8 * 2304 + 8 * 256 + 2048
    R0, Q0, K0, V0, W0 = (0, 8 * 1280, 8 * 2560, 8 * 2560 + 8 * 2304,
                          8 * 2560 + 8 * 2304 + 8 * 256)
    posT = pos.T.astype(bf)                      # [DIM, FULL]
    in_maps = []
    for c in range(8):
        bg, hg = c // 4, c % 4
        sl = slice(hg * HDC, (hg + 1) * HDC)
        bsl = slice(2 * bg, 2 * bg + 2)
        uvec = (np.asarray(u).reshape(-1) + np.asarray(b_q))[sl]
        vvec = (np.asarray(v).reshape(-1) + np.asarray(b_q))[sl]
        # uvp[p, 2t+0/1] = u/v for head-dim t*128+p
        uvp = np.stack([uvec[0:128], vvec[0:128],
                        uvec[128:256], vvec[128:256]], axis=1)
        xf = full_input[:, bsl, :].transpose(2, 1, 0).astype(bf)  # [DIM,2,FULL]
        xc = inputs[:, bsl, :].transpose(2, 1, 0).astype(bf)      # [DIM,2,CUR]
        wq = W_q[:, sl].astype(bf)
        wk = W_kv[:, hg * HDC:(hg + 1) * HDC].astype(bf)
        wv = W_kv[:, H * D + hg * HDC:H * D + (hg + 1) * HDC].astype(bf)
        wpos = W_pos[:, sl].astype(bf)
        wproj = W_proj[sl, :].astype(bf)          # [HDC, DIM]

        P = np.zeros((128, PCOLS), bf)
        for k8 in range(8):
            dsl = slice(k8 * 128, (k8 + 1) * 128)
            P[:, R0 + k8 * 1280:R0 + k8 * 1280 + 1024] = posT[dsl]
            P[:, R0 + k8 * 1280 + 1024:R0 + (k8 + 1) * 1280] = wpos[dsl]
            P[:, Q0 + k8 * 1280:Q0 + k8 * 1280 + 1024] = \
                xc[dsl].reshape(128, NB * CUR)
            P[:, Q0 + k8 * 1280 + 1024:Q0 + (k8 + 1) * 1280] = wq[dsl]
            P[:, K0 + k8 * 2304:K0 + k8 * 2304 + 2048] = \
                xf[dsl].reshape(128, NB * FULL)
            P[:, K0 + k8 * 2304 + 2048:K0 + (k8 + 1) * 2304] = wk[dsl]
            P[:, V0 + k8 * 256:V0 + (k8 + 1) * 256] = wv[dsl]
        P[:, W0:W0 + 1024] = wproj[0:128]
        P[:, W0 + 1024:W0 + 2048] = wproj[128:256]
        in_maps.append({
            "packed": P,
            "uvp": np.ascontiguousarray(uvp).astype(np.float32),
        })

    _CACHED["maps"] = in_maps
    res = run_bass_kernel_spmd(nc, in_maps, list(range(8)))
    out = np.zeros((CUR, BS, DIM), np.float32)
    for c in range(8):
        bg, hg = c // 4, c % 4
        r = res.results[c]["outp"].reshape(CUR, NB, DIM)
        out[:, 2 * bg, :] += r[:, 0, :]
        out[:, 2 * bg + 1, :] += r[:, 1, :]
    return out
